# revision 7
# baseline (speedup 1.0000x reference)
"""Trainium2 Bass kernel for causal MultiHeadAttention + residual + LayerNorm.

Problem shapes (hardcoded):
  B=4, S=2048, D_MODEL=1024, H=8 heads, d_k=128.
  out = LayerNorm(queries + MHA(queries, keys, values))

Sharding (8 cores):
  Launch 1 (attention): core c <-> (batch b = c//2, head group g = c%2 -> heads
  4g..4g+3).  Q/K/V weights column-sharded by head group.
  All big matmuls except QK^T run in fp8e4 DoubleRow perf mode (256-deep
  contraction per pass, 2x PE throughput); QK^T stays bf16 (contraction is
  d_k=128, DoubleRow cannot apply).

  Schedule: the PE stream is software-pipelined so the ScalarE-bound exp
  windows of head h are filled with PE work: V projection (head 0's window),
  P@V of head h-1, rowsum matmuls of head h (emitted as soon as their P
  chunks are exp'd), and K/Q projections of head h+1 at the window tail.
  Rowsums are inverted (DVE reciprocal from PSUM) and partition-broadcast
  via SBUF->SBUF DMA, so O^T is normalized during its PSUM->SBUF drain and
  the attention output DMA'd to DRAM is the final softmax(QK^T)V.

  Launch 2 (layernorm): row-sharded, 1024 rows of the flattened [8192,1024]
  residual per core; pure bf16 add + bn_stats + normalize.
"""

import sys

import numpy as np

for _p in ("/opt/trn_rl_repo", "/opt/pypackages"):
    if _p not in sys.path:
        sys.path.append(_p)

import ml_dtypes  # noqa: E402

import concourse.bass as bass  # noqa: E402
import concourse.mybir as mybir  # noqa: E402
from concourse.tile import TileContext  # noqa: E402
from concourse.tile import add_dep_helper as _adh  # noqa: E402
from concourse.bass_utils import run_bass_kernel_spmd  # noqa: E402
from concourse.masks import make_lower_triangular  # noqa: E402

B = 4
S = 2048
D = 1024
H = 8
DK = 128
HG = 4  # heads per core
NCORES = 8
WS = 32.0  # host-side weight scale so fp8 sees ~N(0,1) values
SCALE = 1.0 / np.sqrt(np.float32(DK))
C_SHIFT = 2.0  # exp(s - C): keeps fp8 P below overflow (TRN e4m3 max 240)
NEG_INF = -1e9
EPS = 1e-6

BF16 = mybir.dt.bfloat16
F32 = mybir.dt.float32
FP8 = mybir.dt.float8e4
NPBF16 = ml_dtypes.bfloat16
NPFP8 = ml_dtypes.float8_e4m3  # IEEE e4m3 (max 240) == TRN FP8_EXP4
DR = mybir.MatmulPerfMode.DoubleRow

KP = D // 256   # 4 contraction pair-chunks (256 rows each)
NS = S // 512   # 4 query ranges of 512
NJ = S // 128   # 16 key chunks of 128
NM = NJ // 2    # 8 key pair-chunks of 256
HW = HG * DK    # 512 columns per head group


def _bcast_rows(ap, n=128):
    """Broadcast a row across n partitions (stride-0 partition dim)."""
    return bass.AP(tensor=ap.tensor, offset=ap.offset, ap=[[0, n]] + list(ap.ap)[1:])


def _dedupe_ldweights(nc):
    """Remove InstLdweights that reload the exact weights already resident in
    the PE array (same AP/perf_mode/tile_position as the previous LDW on the
    PE stream, nothing reloaded between).  All stationary tiles in this
    kernel are write-once, so AP identity implies content identity.  LDWs
    carry no sem updates here, so deletion cannot break downstream waits;
    LDWs that carry waits are kept.  Each deleted LDW saves ~100ns of PE
    sequencer dispatch."""
    n_del = 0
    for f in nc.m.functions:
        for bb in f.blocks:
            il = bb.instructions
            out = []
            pk = None
            changed = False
            for ins in il:
                tname = type(ins).__name__
                if tname == "InstLdweights":
                    si = ins.sync_info
                    has_sync = si is not None and (
                        len(si.on_wait) > 0 or len(si.on_update) > 0
                    )
                    key = (
                        str(ins.ins[0]),
                        str(ins.perf_mode),
                        str(ins.tile_position),
                        str(ins.is_transpose),
                    )
                    if key == pk and not has_sync:
                        n_del += 1
                        changed = True
                        continue
                    pk = key
                elif tname == "InstMatmult" and getattr(ins, "is_transpose", None):
                    pk = None  # transpose clobbers the loaded weights
                out.append(ins)
            if changed:
                il[:] = out
    return n_del


def _split_excess_waits(nc):
    """Workaround for this walrus build: engine (TPB) instructions accept at
    most one sync-wait command (EventSemaphore: two), but Tile attaches one
    wait per dependency.  Move excess waits onto same-engine NOPs inserted
    immediately before the over-limit instruction."""
    n_new = 0
    for f in nc.m.functions:
        for bb in f.blocks:
            il = bb.instructions
            out = []
            changed = False
            for ins in il:
                si = ins.sync_info
                tname = type(ins).__name__
                if si is not None:
                    cap = 2 if tname == "InstEventSemaphore" else 1
                    waits = list(si.on_wait)
                    if len(waits) > cap:
                        for w in waits[cap:]:
                            nop = mybir.InstNoOp(
                                name=f"I-wsplit-{n_new}",
                                sync_info=mybir.SyncInfo(
                                    on_wait=[w], on_update=[]
                                ),
                                bass_nofuse=True,
                                engine=ins.engine,
                            )
                            n_new += 1
                            out.append(nop)
                        si.on_wait = waits[:cap]
                        changed = True
                out.append(ins)
            if changed:
                il[:] = out
    return n_new


def _build_attention():
    """Per-core attention program: 4 heads of one batch, fp8 DoubleRow."""
    nc = bass.Bass()

    # activations pre-chunked on host: [sc, 128, kp, 2, 512] fp8
    xq_t = nc.dram_tensor("xq_t", [NS, 128, KP, 2, 512], FP8, kind="ExternalInput")
    xk_t = nc.dram_tensor("xk_t", [NS, 128, KP, 2, 512], FP8, kind="ExternalInput")
    xv_t = nc.dram_tensor("xv_t", [NS, 128, KP, 2, 512], FP8, kind="ExternalInput")
    # weights pre-permuted+scaled on host: [128, kp, 2, 512] fp8
    wq = nc.dram_tensor("wq", [128, KP, 2, HW], FP8, kind="ExternalInput")
    wk = nc.dram_tensor("wk", [128, KP, 2, HW], FP8, kind="ExternalInput")
    wv = nc.dram_tensor("wv", [128, KP, 2, HW], FP8, kind="ExternalInput")
    # biases packed [bq32 | bk32 | bv_bcast]: [128, HG+HG+HW] f32
    bqkv = nc.dram_tensor("bqkv", [128, 2 * HG + HW], F32, kind="ExternalInput")
    # per-head NORMALIZED attention output O^T (softmax applied in-kernel)
    o_t = nc.dram_tensor("o_t", [HG, DK, S], BF16, kind="ExternalOutput")
    # DRAM scratch for the 1/rowsum partition-broadcast round trip (SBUF
    # sources cannot have stride-0 partition dims in DMA APs)
    rsd = nc.dram_tensor("rsd", [HG, S], BF16, kind="Internal")

    with TileContext(nc) as tc:
        from contextlib import ExitStack

        with ExitStack() as ctx:
            consts = ctx.enter_context(tc.tile_pool(name="consts", bufs=1))
            xpool = ctx.enter_context(tc.tile_pool(name="x", bufs=1))
            wpool = ctx.enter_context(tc.tile_pool(name="w", bufs=1))
            proj_out = ctx.enter_context(tc.tile_pool(name="proj_out", bufs=1))
            ptpool = ctx.enter_context(tc.tile_pool(name="pt", bufs=2))
            osbpool = ctx.enter_context(tc.tile_pool(name="osb", bufs=4))
            ripool = ctx.enter_context(tc.tile_pool(name="ri", bufs=4))
            rbpool = ctx.enter_context(tc.tile_pool(name="rb", bufs=2))
            stpool = ctx.enter_context(
                tc.tile_pool(name="st", bufs=2, space="PSUM")
            )
            rspool = ctx.enter_context(
                tc.tile_pool(name="rsp", bufs=2, space="PSUM")
            )
            otpool = ctx.enter_context(
                tc.tile_pool(name="ot", bufs=1, space="PSUM")
            )

            # --- constants ---
            tril = consts.tile([128, 128], F32)  # additive: -1e9 where k > q
            make_lower_triangular(nc, tril, val=NEG_INF, diag=False)
            # pair-dim stride must be 16B-aligned for dual-fp8 LDWEIGHTS
            ones8 = consts.tile([128, 2, 16], FP8)
            nc.vector.memset(ones8, 1.0)
            negc_sb = consts.tile([128, 1], F32)
            nc.vector.memset(negc_sb, -float(C_SHIFT))
            b_sb = consts.tile([128, 2 * HG + HW], F32)
            # bias DMA split so the first K drain doesn't wait on one big DMA
            nc.scalar.dma_start(out=b_sb[:, 0:2 * HG], in_=bqkv[:, 0:2 * HG])
            nc.scalar.dma_start(
                out=b_sb[:, 2 * HG:2 * HG + 256], in_=bqkv[:, 2 * HG:2 * HG + 256]
            )
            nc.scalar.dma_start(
                out=b_sb[:, 2 * HG + 256:], in_=bqkv[:, 2 * HG + 256:]
            )
            bq_sb = b_sb[:, 0:HG]
            bk_sb = b_sb[:, HG:2 * HG]
            bv_sb = b_sb[:, 2 * HG:]

            # --- SBUF tiles for activations / projections ---
            xq8 = xpool.tile([128, NS, KP, 2, 512], FP8, tag="xq", name="xq")
            xk8 = xpool.tile([128, NS, KP, 2, 512], FP8, tag="xk", name="xk")
            xv8 = xpool.tile([128, NS, KP, 2, 512], FP8, tag="xv", name="xv")
            wq_t = wpool.tile([128, KP, 2, HW], FP8, tag="wq_t", name="wq_t")
            wk_t = wpool.tile([128, KP, 2, HW], FP8, tag="wk_t", name="wk_t")
            wv_t = wpool.tile([128, KP, 2, HW], FP8, tag="wv_t", name="wv_t")
            qt_sb = [proj_out.tile([128, S], BF16, tag=f"qt{h}", name=f"qt{h}")
                     for h in range(HG)]
            kt_sb = [proj_out.tile([128, S], BF16, tag=f"kt{h}", name=f"kt{h}")
                     for h in range(HG)]
            v8_sb = proj_out.tile([128, NJ, HW], FP8, tag="v8", name="v8")

            # --- input DMAs, kp-split for queue parallelism, spread across
            # issue engines so descriptor generation doesn't serialize ---
            for kp in range(KP):
                nc.sync.dma_start(out=wk_t[:, kp], in_=wk[:, kp])
            for sc in range(NS):
                for kp in range(KP):
                    nc.sync.dma_start(out=xk8[:, sc, kp], in_=xk_t[sc, :, kp])
            for kp in range(KP):
                nc.scalar.dma_start(out=wq_t[:, kp], in_=wq[:, kp])
            for sc in range(NS):
                for kp in range(KP):
                    nc.gpsimd.dma_start(out=xq8[:, sc, kp], in_=xq_t[sc, :, kp])
            for kp in range(KP):
                nc.gpsimd.dma_start(out=wv_t[:, kp], in_=wv[:, kp])
            for sc in range(NS):
                for kp in range(KP):
                    nc.gpsimd.dma_start(out=xv8[:, sc, kp], in_=xv_t[sc, :, kp])

            # ---------- emission helpers ----------
            bases = [512 * (m // 2) for m in range(NM)]
            pts_h = {}
            rb_h = {}   # (h, r) -> broadcast 1/rowsum tile [128, 512] bf16

            def emit_proj_sc(h, sc, w_t, b_col, out_sb):
                """One 512-query chunk of a K^T/Q^T projection (kp inner, so
                it can start as soon as that sc's activations land)."""
                ps = rspool.tile([128, 512], F32, tag="rsp", name=f"pj{h}{sc}")
                for kp in range(KP):
                    nc.tensor.matmul(
                        ps,
                        lhsT=w_t[:, kp, :, h * DK:(h + 1) * DK],
                        rhs=xk8[:, sc, kp] if w_t is wk_t else xq8[:, sc, kp],
                        start=(kp == 0),
                        stop=(kp == KP - 1),
                        perf_mode=DR,
                    )
                nc.vector.tensor_scalar_add(
                    out=out_sb[:, sc * 512:(sc + 1) * 512],
                    in0=ps,
                    scalar1=b_col[:, h:h + 1],
                )

            def emit_proj_block(h, w_t, x_t8, b_col, out_sb):
                """K^T/Q^T projection for one head, kp outer (4 LDWs)."""
                pss = [stpool.tile([128, 1024], F32, tag="st", name=f"ps{h}")
                       for _ in range(2)]
                for kp in range(KP):
                    for sc in range(4):
                        nc.tensor.matmul(
                            pss[sc // 2][:, (sc % 2) * 512:(sc % 2 + 1) * 512],
                            lhsT=w_t[:, kp, :, h * DK:(h + 1) * DK],
                            rhs=x_t8[:, sc, kp],
                            start=(kp == 0),
                            stop=(kp == KP - 1),
                            perf_mode=DR,
                        )
                for st2 in range(2):
                    nc.vector.tensor_scalar_add(
                        out=out_sb[:, st2 * 1024:(st2 + 1) * 1024],
                        in0=pss[st2],
                        scalar1=b_col[:, h:h + 1],
                    )

            def v_unit(sc, tt):
                """V projection chunk: V[sc queries, cols] for one 256-row
                slab pair; fp8 output at true scale."""
                ps = stpool.tile([128, 1024], F32, tag="st", name="psv")
                for sbl in range(2):
                    sbl2 = 2 * tt + sbl
                    for kp in range(KP):
                        nc.tensor.matmul(
                            ps[:, sbl * 512:(sbl + 1) * 512],
                            lhsT=xv8[:, sc, kp, :,
                                     sbl2 * 128:(sbl2 + 1) * 128],
                            rhs=wv_t[:, kp],
                            start=(kp == 0),
                            stop=(kp == KP - 1),
                            perf_mode=DR,
                        )
                for sbl in range(2):
                    sb = 4 * sc + 2 * tt + sbl
                    nc.vector.scalar_tensor_tensor(
                        out=v8_sb[:, sb, :],
                        in0=ps[:, sbl * 512:(sbl + 1) * 512],
                        scalar=1.0 / WS,
                        in1=bv_sb,
                        op0=mybir.AluOpType.mult,
                        op1=mybir.AluOpType.add,
                    )

            def emit_pts_alloc(h):
                pts = []
                for m in range(NM):
                    pt = ptpool.tile([128, 2, S - bases[m]], FP8,
                                     tag=f"ptp{m}", name=f"pt{h}_{m}")
                    pts.append(pt)
                pts_h[h] = pts
                # zero the causally-invalid diagonal blocks so DoubleRow
                # P@V / rowsum matmuls can run unmasked over full pairs
                for r in range(NS):
                    nc.vector.memset(pts[2 * r][:, 1, 0:128], 0.0)
                    nc.vector.memset(pts[2 * r + 1][:, :, 0:384], 0.0)

            def a_unit(h, j, hl):
                """Scores for key chunk j, query cols [hl*1024,(hl+1)*1024),
                plus the exp into fp8 P^T."""
                pts = pts_h[h]
                m = j // 2
                jq = j * 128
                base = bases[m]
                qlo = max(hl * 1024, jq)
                a = qlo - hl * 1024
                r0 = j // 4
                st = stpool.tile([128, 1024], F32, tag="st", name="st")
                for r in range(max(2 * hl, r0), 2 * hl + 2):
                    rqlo = max(r * 512, jq)
                    ra = rqlo - hl * 1024
                    nc.tensor.matmul(
                        st[:, ra:(r + 1) * 512 - hl * 1024],
                        lhsT=kt_sb[h][:, jq:jq + 128],
                        rhs=qt_sb[h][:, rqlo:(r + 1) * 512],
                        start=True,
                        stop=True,
                    )
                if qlo == jq:
                    nc.vector.tensor_add(
                        out=st[:, a:a + 128],
                        in0=st[:, a:a + 128],
                        in1=tril,
                    )
                nc.scalar.activation(
                    out=pts[m][:, j % 2, qlo - base:(hl + 1) * 1024 - base],
                    in_=st[:, a:1024],
                    func=mybir.ActivationFunctionType.Exp,
                    scale=float(SCALE / (WS * WS)),
                    bias=negc_sb,
                )

            def rs_unit(h, r):
                """Rowsums for query range r (fp8 DoubleRow ones-matmul over
                all P chunks), then 1/rowsum broadcast across partitions so
                the O^T drain can normalize in place."""
                pts = pts_h[h]
                rsp_pk = rspool.tile([128, 512], F32, tag="rsp",
                                     name=f"rsp{h}_{r}")
                for m in range(2 * r + 2):
                    nc.tensor.matmul(
                        rsp_pk[0:1, :],
                        lhsT=ones8[:, :, 0:1],
                        rhs=pts[m][:, :, r * 512 - bases[m]:
                                   (r + 1) * 512 - bases[m]],
                        start=(m == 0),
                        stop=(m == 2 * r + 1),
                        perf_mode=DR,
                    )
                ri = ripool.tile([1, 512], BF16, tag="ri", name=f"ri{h}_{r}")
                with nc.allow_low_precision(
                    reason="bf16 1/rowsum: ~0.4% on attn term vs 2e-2 gate"
                ):
                    nc.vector.reciprocal(out=ri, in_=rsp_pk[0:1, :])
                rs_slice = rsd[h:h + 1, r * 512:(r + 1) * 512]
                d_wr = nc.sync.dma_start(out=rs_slice, in_=ri)
                rb = rbpool.tile([128, 512], BF16, tag=f"rb{r}",
                                 name=f"rb{h}_{r}")
                d_rd = nc.sync.dma_start(out=rb, in_=_bcast_rows(rs_slice))
                _adh(d_rd.ins, d_wr.ins, reason="rowsum bcast RAW via DRAM")
                rb_h[(h, r)] = rb

            def b_units(h):
                """P@V for head h as a list of (cost_ns, closure) filler
                units: per-m matmul groups + normalized drains."""
                pts = pts_h[h]
                units = []
                state = {}

                def mk_group(half, m):
                    def fn():
                        if m == 0:
                            state[half] = otpool.tile(
                                [128, 1024], F32, tag="ot", name=f"ot{h}_{half}"
                            )
                        ot_ps = state[half]
                        for k in range(2):
                            r = 2 * half + k
                            if m >= 2 * r + 2:
                                continue
                            nc.tensor.matmul(
                                ot_ps[:, k * 512:(k + 1) * 512],
                                lhsT=v8_sb[:, 2 * m:2 * m + 2,
                                           h * DK:(h + 1) * DK],
                                rhs=pts[m][:, :, r * 512 - bases[m]:
                                           (r + 1) * 512 - bases[m]],
                                start=(m == 0),
                                stop=(m == 2 * r + 1),
                                perf_mode=DR,
                            )
                    return fn

                def mk_drain(half):
                    def fn():
                        ot_ps = state[half]
                        for k in range(2):
                            r = 2 * half + k
                            o_sb = osbpool.tile([128, 512], BF16, tag="osb",
                                                name=f"o_sb{h}_{r}")
                            with nc.allow_low_precision(
                                reason="bf16 attn out: ~0.4% vs 2e-2 gate"
                            ):
                                nc.vector.tensor_mul(
                                    out=o_sb, in0=ot_ps[:, k * 512:(k + 1) * 512],
                                    in1=rb_h.pop((h, r)),
                                )
                            for cs in range(2):
                                nc.gpsimd.dma_start(
                                    out=o_t[h, :, r * 512 + cs * 256:
                                            r * 512 + (cs + 1) * 256],
                                    in_=o_sb[:, cs * 256:(cs + 1) * 256],
                                )
                    return fn

                for half in range(2):
                    nm_half = 2 * (2 * half + 1) + 2
                    for m in range(nm_half):
                        n_mm = sum(1 for k in range(2)
                                   if m < 2 * (2 * half + k) + 2)
                        units.append((150 + 213 * n_mm + 135, mk_group(half, m)))
                    units.append((100, mk_drain(half)))
                return units

            # ---------- emission schedule ----------
            # K0/Q0 projections in sc-chunks so the PE starts as soon as the
            # first kp-split DMAs land, and A(0) can begin before xv arrives.
            for sc in range(NS):
                emit_proj_sc(0, sc, wk_t, bk_sb, kt_sb[0])
                emit_proj_sc(0, sc, wq_t, bq_sb, qt_sb[0])

            for h in range(HG):
                emit_pts_alloc(h)
                # filler units woven between A chunks: V projection during
                # head 0, P@V of head h-1 afterwards
                if h == 0:
                    fillers = [(2800, (lambda sc=sc, tt=tt:
                                       v_unit(sc, tt)))
                               for sc in range(NS) for tt in range(2)]
                else:
                    fillers = b_units(h - 1)
                total_fill = sum(c for c, _ in fillers) or 1
                fillers = list(fillers)
                balance = 0.0
                emitted = 0.0
                # ScalarE cost of each A unit, to pace fillers linearly
                a_units = []
                for j in range(NJ):
                    for hl in range(j // 8, 2):
                        qlo = max(hl * 1024, j * 128)
                        a_units.append((j, hl, (1024 * (hl + 1) - qlo) * 0.72
                                        + 260))
                total_a = sum(c for _, _, c in a_units)
                for j, hl, cost in a_units:
                    a_unit(h, j, hl)
                    emitted += cost
                    # rowsums as soon as all P chunks for range r are exp'd
                    if hl == 1 and (j + 1) % 4 == 0:
                        rs_unit(h, (j + 1) // 4 - 1)
                    want = total_fill * emitted / total_a
                    while fillers and balance < want:
                        c, fn = fillers.pop(0)
                        fn()
                        balance += c
                for c, fn in fillers:
                    fn()
                if h + 1 < HG:
                    emit_proj_block(h + 1, wk_t, xk8, bk_sb, kt_sb[h + 1])
                    emit_proj_block(h + 1, wq_t, xq8, bq_sb, qt_sb[h + 1])
            # tail: P@V of the last head
            for c, fn in b_units(HG - 1):
                fn()
    _dedupe_ldweights(nc)
    _split_excess_waits(nc)
    return nc


def _build_layernorm(affine=True):
    """Per-core: residual add + LayerNorm over 1024 rows of [8192, 1024].

    Inputs arrive bf16 and already normalized (softmax applied in the
    attention kernel).  Everything stays bf16 so DVE runs in 2x mode.
    affine=False omits gamma/beta (valid when gamma==1, beta==0)."""
    nc = bass.Bass()
    RPC = (B * S) // NCORES  # 1024 rows per core

    attn = nc.dram_tensor("attn", [RPC, D], BF16, kind="ExternalInput")
    resid = nc.dram_tensor("resid", [RPC, D], BF16, kind="ExternalInput")
    gamma = nc.dram_tensor("gamma", [D], F32, kind="ExternalInput")
    beta = nc.dram_tensor("beta", [D], F32, kind="ExternalInput")
    out = nc.dram_tensor("out", [RPC, D], BF16, kind="ExternalOutput")

    with TileContext(nc) as tc:
        with (
            tc.tile_pool(name="consts", bufs=1) as consts,
            tc.tile_pool(name="work", bufs=3) as work,
            tc.tile_pool(name="stat", bufs=4) as statp,
        ):
            if affine:
                gamma_sb = consts.tile([128, D], BF16)
                beta_sb = consts.tile([128, D], BF16)
                nc.gpsimd.dma_start(
                    out=gamma_sb,
                    in_=bass.AP(tensor=gamma[:].tensor, offset=gamma[:].offset,
                                ap=[[0, 128]] + list(gamma[:].ap)),
                )
                nc.gpsimd.dma_start(
                    out=beta_sb,
                    in_=bass.AP(tensor=beta[:].tensor, offset=beta[:].offset,
                                ap=[[0, 128]] + list(beta[:].ap)),
                )
            eps_sb = consts.tile([128, 1], F32)
            nc.vector.memset(eps_sb, EPS)

            nsub = D // 512  # bn_stats free-dim limit
            NT = RPC // 128
            for t in range(NT):
                ab = work.tile([128, D], BF16, tag="ab", name="ab")
                rb = work.tile([128, D], BF16, tag="rb", name="rb")
                x = work.tile([128, D], BF16, tag="x", name="x")
                nc.sync.dma_start(out=ab, in_=attn[t * 128:(t + 1) * 128, :])
                nc.sync.dma_start(out=rb, in_=resid[t * 128:(t + 1) * 128, :])
                with nc.allow_low_precision(
                    reason="bf16 residual add: ~0.4% vs 2e-2 gate"
                ):
                    nc.vector.tensor_add(out=x, in0=ab, in1=rb)

                stats = statp.tile([128, nsub, 6], F32, tag="stats",
                                   name="stats")
                for sgi in range(nsub):
                    nc.vector.bn_stats(
                        out=stats[:, sgi, :],
                        in_=x[:, sgi * 512:(sgi + 1) * 512],
                    )
                mv = statp.tile([128, 2], F32, tag="mv", name="mv")
                nc.vector.bn_aggr(out=mv, in_=stats)
                rstd = statp.tile([128, 1], F32, tag="rstd", name="rstd")
                nc.scalar.activation(
                    out=rstd,
                    in_=mv[:, 1:2],
                    func=mybir.ActivationFunctionType.Sqrt,
                    bias=eps_sb,
                    scale=1.0,
                )
                nc.vector.reciprocal(out=rstd, in_=rstd)
                xo = work.tile([128, D], BF16, tag="xo", name="xo")
                with nc.allow_low_precision(
                    reason="bf16 LN output: ~0.2% rounding vs 2e-2 gate"
                ):
                    nc.vector.tensor_scalar(
                        out=xo if not affine else x,
                        in0=x,
                        scalar1=mv[:, 0:1],
                        scalar2=rstd,
                        op0=mybir.AluOpType.subtract,
                        op1=mybir.AluOpType.mult,
                    )
                    if affine:
                        nc.vector.tensor_mul(out=x, in0=x, in1=gamma_sb)
                        nc.vector.tensor_add(out=xo, in0=x, in1=beta_sb)
                nc.gpsimd.dma_start(
                    out=out[t * 128:(t + 1) * 128, :], in_=xo
                )
    _split_excess_waits(nc)
    return nc


_CACHE = {}


def _get_programs(affine=True):
    # note: walrus's --enable-ldw-opt=true rejects DoubleRow LDWEIGHTS
    # ("InstLdweights is not compatible with LDW optimization"), so redundant
    # weight loads are removed by _dedupe_ldweights instead.
    if "attn" not in _CACHE:
        _CACHE["attn"] = _build_attention()
    key = ("ln", affine)
    if key not in _CACHE:
        _CACHE[key] = _build_layernorm(affine=affine)
    return _CACHE["attn"], _CACHE[key]


def _prep_x(xb):
    """[S, D] f32 -> [sc, 128, kp, 2, 512] fp8 with
    x8[sc, p, kp, i, s'] = X[sc*512+s', 256*kp+128*i+p]."""
    xT = np.asarray(xb, dtype=np.float32).T  # [D, S]
    arr = xT.reshape(KP, 2, 128, NS, 512).transpose(3, 2, 0, 1, 4)
    return np.ascontiguousarray(arr.astype(NPFP8))


def _prep_w(Wm, g):
    """W[:, g*512:(g+1)*512]*WS -> [128, kp, 2, 512] fp8."""
    ws = np.asarray(Wm, dtype=np.float32)[:, g * HW:(g + 1) * HW] * WS
    arr = ws.reshape(KP, 2, 128, HW).transpose(2, 0, 1, 3)
    return np.ascontiguousarray(arr.astype(NPFP8))


def _run(inputs, trace=False):
    """Returns (output, attn_results, ln_results)."""
    gamma_np = np.asarray(inputs["gamma"], dtype=np.float32)
    beta_np = np.asarray(inputs["beta"], dtype=np.float32)
    affine = not (np.all(gamma_np == 1.0) and np.all(beta_np == 0.0))
    nc_attn, nc_ln = _get_programs(affine=affine)

    q = np.ascontiguousarray(np.asarray(inputs["queries"], dtype=np.float32))
    k = np.ascontiguousarray(np.asarray(inputs["keys"], dtype=np.float32))
    v = np.ascontiguousarray(np.asarray(inputs["values"], dtype=np.float32))
    Wq = np.asarray(inputs["Wq"], dtype=np.float32)
    Wk = np.asarray(inputs["Wk"], dtype=np.float32)
    Wv = np.asarray(inputs["Wv"], dtype=np.float32)
    bq = np.asarray(inputs["bq"], dtype=np.float32)
    bk = np.asarray(inputs["bk"], dtype=np.float32)
    bv = np.asarray(inputs["bv"], dtype=np.float32)

    xt = {}
    for b in range(B):
        xt[("q", b)] = _prep_x(q[b])
        xt[("k", b)] = _prep_x(k[b])
        xt[("v", b)] = _prep_x(v[b])
    wslices = {}
    bslices = {}
    for g in range(2):
        cols = slice(g * HW, (g + 1) * HW)
        wslices[("q", g)] = _prep_w(Wq, g)
        wslices[("k", g)] = _prep_w(Wk, g)
        wslices[("v", g)] = _prep_w(Wv, g)
        bslices[g] = np.ascontiguousarray(np.concatenate([
            (WS * bq[cols]).reshape(HG, 128).T,
            (WS * bk[cols]).reshape(HG, 128).T,
            np.broadcast_to(bv[cols], (128, HW)),
        ], axis=1, dtype=np.float32))

    in_maps = []
    for c in range(NCORES):
        b, g = c // 2, c % 2
        in_maps.append({
            "xq_t": xt[("q", b)],
            "xk_t": xt[("k", b)],
            "xv_t": xt[("v", b)],
            "wq": wslices[("q", g)],
            "wk": wslices[("k", g)],
            "wv": wslices[("v", g)],
            "bqkv": bslices[g],
        })

    res1 = run_bass_kernel_spmd(
        nc_attn, in_maps, core_ids=list(range(NCORES)), trace=trace
    )

    # assemble attention output [B, S, D] bf16 (already normalized in-kernel)
    attn_full = np.empty((B, S, D), dtype=NPBF16)
    for c in range(NCORES):
        b, g = c // 2, c % 2
        ot = res1.results[c]["o_t"]  # [HG, DK, S]
        for i in range(HG):
            attn_full[b, :, (g * HG + i) * DK:(g * HG + i + 1) * DK] = ot[i].T

    attn_flat = attn_full.reshape(B * S, D)
    q_flat = q.reshape(B * S, D).astype(NPBF16)
    RPC = (B * S) // NCORES
    in_maps2 = []
    for c in range(NCORES):
        rows = slice(c * RPC, (c + 1) * RPC)
        in_maps2.append({
            "attn": np.ascontiguousarray(attn_flat[rows]),
            "resid": np.ascontiguousarray(q_flat[rows]),
            "gamma": gamma_np,
            "beta": beta_np,
        })
    res2 = run_bass_kernel_spmd(
        nc_ln, in_maps2, core_ids=list(range(NCORES)), trace=trace
    )
    out = np.concatenate(
        [res2.results[c]["out"].astype(np.float32) for c in range(NCORES)],
        axis=0,
    ).reshape(B, S, D)
    return out, res1, res2


def kernel(**inputs):
    out, _, _ = _run(inputs, trace=False)
    return out


# revision 11
# speedup vs baseline: 1.0157x; 1.0157x over previous
"""Trainium2 Bass kernel for causal MultiHeadAttention + residual + LayerNorm.

Problem shapes (hardcoded):
  B=4, S=2048, D_MODEL=1024, H=8 heads, d_k=128.
  out = LayerNorm(queries + MHA(queries, keys, values))

Sharding (8 cores):
  Launch 1 (attention): core c <-> (batch b = c//2, head group g = c%2 -> heads
  4g..4g+3).  Q/K/V weights column-sharded by head group.
  All big matmuls except QK^T run in fp8e4 DoubleRow perf mode (256-deep
  contraction per pass, 2x PE throughput); QK^T stays bf16 (contraction is
  d_k=128, DoubleRow cannot apply).

  Schedule: the PE stream is software-pipelined so the ScalarE-bound exp
  windows of head h are filled with PE work: V projection (head 0's window),
  P@V of head h-1, rowsum matmuls of head h (emitted as soon as their P
  chunks are exp'd), and K/Q projections of head h+1 at the window tail.
  Rowsums are inverted (DVE reciprocal from PSUM) and partition-broadcast
  via SBUF->SBUF DMA, so O^T is normalized during its PSUM->SBUF drain and
  the attention output DMA'd to DRAM is the final softmax(QK^T)V.

  Launch 2 (layernorm): row-sharded, 1024 rows of the flattened [8192,1024]
  residual per core; pure bf16 add + bn_stats + normalize.
"""

import sys

import numpy as np

for _p in ("/opt/trn_rl_repo", "/opt/pypackages"):
    if _p not in sys.path:
        sys.path.append(_p)

import ml_dtypes  # noqa: E402

import concourse.bass as bass  # noqa: E402
import concourse.mybir as mybir  # noqa: E402
from concourse.tile import TileContext  # noqa: E402
from concourse.tile import add_dep_helper as _adh  # noqa: E402
from concourse.bass_utils import run_bass_kernel_spmd  # noqa: E402
from concourse.masks import make_lower_triangular  # noqa: E402

B = 4
S = 2048
D = 1024
H = 8
DK = 128
HG = 4  # heads per core
NCORES = 8
WS = 32.0  # host-side weight scale so fp8 sees ~N(0,1) values
SCALE = 1.0 / np.sqrt(np.float32(DK))
C_SHIFT = 2.0  # exp(s - C): keeps fp8 P below overflow (TRN e4m3 max 240)
NEG_INF = -1e9
EPS = 1e-6

BF16 = mybir.dt.bfloat16
F32 = mybir.dt.float32
FP8 = mybir.dt.float8e4
NPBF16 = ml_dtypes.bfloat16
NPFP8 = ml_dtypes.float8_e4m3  # IEEE e4m3 (max 240) == TRN FP8_EXP4
DR = mybir.MatmulPerfMode.DoubleRow

KP = D // 256   # 4 contraction pair-chunks (256 rows each)
NS = S // 512   # 4 query ranges of 512
NJ = S // 128   # 16 key chunks of 128
NM = NJ // 2    # 8 key pair-chunks of 256
HW = HG * DK    # 512 columns per head group


def _bcast_rows(ap, n=128):
    """Broadcast a row across n partitions (stride-0 partition dim)."""
    return bass.AP(tensor=ap.tensor, offset=ap.offset, ap=[[0, n]] + list(ap.ap)[1:])


def _dedupe_ldweights(nc):
    """Remove InstLdweights that reload the exact weights already resident in
    the PE array (same AP/perf_mode/tile_position as the previous LDW on the
    PE stream, nothing reloaded between).  All stationary tiles in this
    kernel are write-once, so AP identity implies content identity.  LDWs
    carry no sem updates here, so deletion cannot break downstream waits;
    LDWs that carry waits are kept.  Each deleted LDW saves ~100ns of PE
    sequencer dispatch."""
    n_del = 0
    for f in nc.m.functions:
        for bb in f.blocks:
            il = bb.instructions
            out = []
            pk = None
            changed = False
            for ins in il:
                tname = type(ins).__name__
                if tname == "InstLdweights":
                    si = ins.sync_info
                    has_sync = si is not None and (
                        len(si.on_wait) > 0 or len(si.on_update) > 0
                    )
                    key = (
                        str(ins.ins[0]),
                        str(ins.perf_mode),
                        str(ins.tile_position),
                        str(ins.is_transpose),
                    )
                    if key == pk and not has_sync:
                        n_del += 1
                        changed = True
                        continue
                    pk = key
                elif tname == "InstMatmult" and getattr(ins, "is_transpose", None):
                    pk = None  # transpose clobbers the loaded weights
                out.append(ins)
            if changed:
                il[:] = out
    return n_del


def _split_excess_waits(nc):
    """Workaround for this walrus build: engine (TPB) instructions accept at
    most one sync-wait command (EventSemaphore: two), but Tile attaches one
    wait per dependency.  Move excess waits onto same-engine NOPs inserted
    immediately before the over-limit instruction."""
    n_new = 0
    for f in nc.m.functions:
        for bb in f.blocks:
            il = bb.instructions
            out = []
            changed = False
            for ins in il:
                si = ins.sync_info
                tname = type(ins).__name__
                if si is not None:
                    cap = 2 if tname == "InstEventSemaphore" else 1
                    waits = list(si.on_wait)
                    if len(waits) > cap:
                        for w in waits[cap:]:
                            nop = mybir.InstNoOp(
                                name=f"I-wsplit-{n_new}",
                                sync_info=mybir.SyncInfo(
                                    on_wait=[w], on_update=[]
                                ),
                                bass_nofuse=True,
                                engine=ins.engine,
                            )
                            n_new += 1
                            out.append(nop)
                        si.on_wait = waits[:cap]
                        changed = True
                out.append(ins)
            if changed:
                il[:] = out
    return n_new


def _build_attention():
    """Per-core attention program: 4 heads of one batch, fp8 DoubleRow."""
    nc = bass.Bass()

    # activations pre-chunked on host: [sc, kp, 128, 2, 512] fp8 so each
    # (sc, kp) DMA piece is fully contiguous (fast descriptor generation)
    xq_t = nc.dram_tensor("xq_t", [NS, KP, 128, 2, 512], FP8, kind="ExternalInput")
    xk_t = nc.dram_tensor("xk_t", [NS, KP, 128, 2, 512], FP8, kind="ExternalInput")
    xv_t = nc.dram_tensor("xv_t", [NS, KP, 128, 2, 512], FP8, kind="ExternalInput")
    # weights pre-permuted+scaled on host: [128, kp, 2, 512] fp8
    wq = nc.dram_tensor("wq", [128, KP, 2, HW], FP8, kind="ExternalInput")
    wk = nc.dram_tensor("wk", [128, KP, 2, HW], FP8, kind="ExternalInput")
    wv = nc.dram_tensor("wv", [128, KP, 2, HW], FP8, kind="ExternalInput")
    # biases packed [bq32 | bk32 | bv_bcast]: [128, HG+HG+HW] f32
    bqkv = nc.dram_tensor("bqkv", [128, 2 * HG + HW], F32, kind="ExternalInput")
    # per-head NORMALIZED attention output O^T (softmax applied in-kernel)
    o_t = nc.dram_tensor("o_t", [HG, DK, S], BF16, kind="ExternalOutput")
    # DRAM scratch for the 1/rowsum partition-broadcast round trip (SBUF
    # sources cannot have stride-0 partition dims in DMA APs)
    rsd = nc.dram_tensor("rsd", [HG, S], BF16, kind="Internal")

    with TileContext(nc) as tc:
        from contextlib import ExitStack

        with ExitStack() as ctx:
            consts = ctx.enter_context(tc.tile_pool(name="consts", bufs=1))
            xpool = ctx.enter_context(tc.tile_pool(name="x", bufs=1))
            wpool = ctx.enter_context(tc.tile_pool(name="w", bufs=1))
            proj_out = ctx.enter_context(tc.tile_pool(name="proj_out", bufs=1))
            ptpool = ctx.enter_context(tc.tile_pool(name="pt", bufs=2))
            osbpool = ctx.enter_context(tc.tile_pool(name="osb", bufs=4))
            ripool = ctx.enter_context(tc.tile_pool(name="ri", bufs=4))
            rbpool = ctx.enter_context(tc.tile_pool(name="rb", bufs=2))
            stpool = ctx.enter_context(
                tc.tile_pool(name="st", bufs=2, space="PSUM")
            )
            rspool = ctx.enter_context(
                tc.tile_pool(name="rsp", bufs=2, space="PSUM")
            )
            otpool = ctx.enter_context(
                tc.tile_pool(name="ot", bufs=1, space="PSUM")
            )

            # --- constants ---
            tril = consts.tile([128, 128], F32)  # additive: -1e9 where k > q
            make_lower_triangular(nc, tril, val=NEG_INF, diag=False)
            # pair-dim stride must be 16B-aligned for dual-fp8 LDWEIGHTS
            ones8 = consts.tile([128, 2, 16], FP8)
            nc.vector.memset(ones8, 1.0)
            negc_sb = consts.tile([128, 1], F32)
            nc.vector.memset(negc_sb, -float(C_SHIFT))
            b_sb = consts.tile([128, 2 * HG + HW], F32)
            # bias DMA split so the first K drain doesn't wait on one big DMA
            nc.scalar.dma_start(out=b_sb[:, 0:2 * HG], in_=bqkv[:, 0:2 * HG])
            nc.scalar.dma_start(
                out=b_sb[:, 2 * HG:2 * HG + 256], in_=bqkv[:, 2 * HG:2 * HG + 256]
            )
            nc.scalar.dma_start(
                out=b_sb[:, 2 * HG + 256:], in_=bqkv[:, 2 * HG + 256:]
            )
            bq_sb = b_sb[:, 0:HG]
            bk_sb = b_sb[:, HG:2 * HG]
            bv_sb = b_sb[:, 2 * HG:]

            # --- SBUF tiles for activations / projections ---
            xq8 = xpool.tile([128, NS, KP, 2, 512], FP8, tag="xq", name="xq")
            xk8 = xpool.tile([128, NS, KP, 2, 512], FP8, tag="xk", name="xk")
            xv8 = xpool.tile([128, NS, KP, 2, 512], FP8, tag="xv", name="xv")
            wq_t = wpool.tile([128, KP, 2, HW], FP8, tag="wq_t", name="wq_t")
            wk_t = wpool.tile([128, KP, 2, HW], FP8, tag="wk_t", name="wk_t")
            wv_t = wpool.tile([128, KP, 2, HW], FP8, tag="wv_t", name="wv_t")
            qt_sb = [proj_out.tile([128, S], BF16, tag=f"qt{h}", name=f"qt{h}")
                     for h in range(HG)]
            kt_sb = [proj_out.tile([128, S], BF16, tag=f"kt{h}", name=f"kt{h}")
                     for h in range(HG)]
            v8_sb = proj_out.tile([128, NJ, HW], FP8, tag="v8", name="v8")

            # --- input DMAs, kp-split for queue parallelism, spread across
            # issue engines so descriptor generation doesn't serialize ---
            for kp in range(KP):
                nc.sync.dma_start(out=wk_t[:, kp], in_=wk[:, kp])
            for sc in range(NS):
                for kp in range(KP):
                    nc.sync.dma_start(out=xk8[:, sc, kp], in_=xk_t[sc, kp])
            for kp in range(KP):
                nc.scalar.dma_start(out=wq_t[:, kp], in_=wq[:, kp])
            for sc in range(NS):
                for kp in range(KP):
                    nc.gpsimd.dma_start(out=xq8[:, sc, kp], in_=xq_t[sc, kp])
            for kp in range(KP):
                nc.gpsimd.dma_start(out=wv_t[:, kp], in_=wv[:, kp])
            for sc in range(NS):
                for kp in range(KP):
                    nc.gpsimd.dma_start(out=xv8[:, sc, kp], in_=xv_t[sc, kp])

            # ---------- emission helpers ----------
            bases = [512 * (m // 2) for m in range(NM)]
            pts_h = {}
            rb_h = {}   # (h, r) -> broadcast 1/rowsum tile [128, 512] bf16

            def emit_proj_sc(h, sc, w_t, b_col, out_sb):
                """One 512-query chunk of a K^T/Q^T projection (kp inner, so
                it can start as soon as that sc's activations land)."""
                ps = rspool.tile([128, 512], F32, tag="rsp", name=f"pj{h}{sc}")
                for kp in range(KP):
                    nc.tensor.matmul(
                        ps,
                        lhsT=w_t[:, kp, :, h * DK:(h + 1) * DK],
                        rhs=xk8[:, sc, kp] if w_t is wk_t else xq8[:, sc, kp],
                        start=(kp == 0),
                        stop=(kp == KP - 1),
                        perf_mode=DR,
                    )
                nc.vector.tensor_scalar_add(
                    out=out_sb[:, sc * 512:(sc + 1) * 512],
                    in0=ps,
                    scalar1=b_col[:, h:h + 1],
                )

            def emit_proj_block(h, w_t, x_t8, b_col, out_sb):
                """K^T/Q^T projection for one head, kp outer (4 LDWs)."""
                pss = [stpool.tile([128, 1024], F32, tag="st", name=f"ps{h}")
                       for _ in range(2)]
                for kp in range(KP):
                    for sc in range(4):
                        nc.tensor.matmul(
                            pss[sc // 2][:, (sc % 2) * 512:(sc % 2 + 1) * 512],
                            lhsT=w_t[:, kp, :, h * DK:(h + 1) * DK],
                            rhs=x_t8[:, sc, kp],
                            start=(kp == 0),
                            stop=(kp == KP - 1),
                            perf_mode=DR,
                        )
                for st2 in range(2):
                    nc.vector.tensor_scalar_add(
                        out=out_sb[:, st2 * 1024:(st2 + 1) * 1024],
                        in0=pss[st2],
                        scalar1=b_col[:, h:h + 1],
                    )

            def v_unit(sc, tt):
                """V projection chunk: V[sc queries, cols] for one 256-row
                slab pair; fp8 output at true scale."""
                ps = stpool.tile([128, 1024], F32, tag="st", name="psv")
                for sbl in range(2):
                    sbl2 = 2 * tt + sbl
                    for kp in range(KP):
                        nc.tensor.matmul(
                            ps[:, sbl * 512:(sbl + 1) * 512],
                            lhsT=xv8[:, sc, kp, :,
                                     sbl2 * 128:(sbl2 + 1) * 128],
                            rhs=wv_t[:, kp],
                            start=(kp == 0),
                            stop=(kp == KP - 1),
                            perf_mode=DR,
                        )
                for sbl in range(2):
                    sb = 4 * sc + 2 * tt + sbl
                    nc.vector.scalar_tensor_tensor(
                        out=v8_sb[:, sb, :],
                        in0=ps[:, sbl * 512:(sbl + 1) * 512],
                        scalar=1.0 / WS,
                        in1=bv_sb,
                        op0=mybir.AluOpType.mult,
                        op1=mybir.AluOpType.add,
                    )

            def emit_pts_alloc(h):
                pts = []
                for m in range(NM):
                    pt = ptpool.tile([128, 2, S - bases[m]], FP8,
                                     tag=f"ptp{m}", name=f"pt{h}_{m}")
                    pts.append(pt)
                pts_h[h] = pts
                # zero the causally-invalid diagonal blocks so DoubleRow
                # P@V / rowsum matmuls can run unmasked over full pairs
                for r in range(NS):
                    nc.vector.memset(pts[2 * r][:, 1, 0:128], 0.0)
                    nc.vector.memset(pts[2 * r + 1][:, :, 0:384], 0.0)

            def a_unit(h, j, hl):
                """Scores for key chunk j, query cols [hl*1024,(hl+1)*1024),
                plus the exp into fp8 P^T."""
                pts = pts_h[h]
                m = j // 2
                jq = j * 128
                base = bases[m]
                qlo = max(hl * 1024, jq)
                a = qlo - hl * 1024
                r0 = j // 4
                st = stpool.tile([128, 1024], F32, tag="st", name="st")
                for r in range(max(2 * hl, r0), 2 * hl + 2):
                    rqlo = max(r * 512, jq)
                    ra = rqlo - hl * 1024
                    nc.tensor.matmul(
                        st[:, ra:(r + 1) * 512 - hl * 1024],
                        lhsT=kt_sb[h][:, jq:jq + 128],
                        rhs=qt_sb[h][:, rqlo:(r + 1) * 512],
                        start=True,
                        stop=True,
                    )
                if qlo == jq:
                    nc.vector.tensor_add(
                        out=st[:, a:a + 128],
                        in0=st[:, a:a + 128],
                        in1=tril,
                    )
                nc.scalar.activation(
                    out=pts[m][:, j % 2, qlo - base:(hl + 1) * 1024 - base],
                    in_=st[:, a:1024],
                    func=mybir.ActivationFunctionType.Exp,
                    scale=float(SCALE / (WS * WS)),
                    bias=negc_sb,
                )

            def rs_unit(h, r):
                """Rowsums for query range r (fp8 DoubleRow ones-matmul over
                all P chunks), then 1/rowsum broadcast across partitions so
                the O^T drain can normalize in place."""
                pts = pts_h[h]
                rsp_pk = rspool.tile([128, 512], F32, tag="rsp",
                                     name=f"rsp{h}_{r}")
                for m in range(2 * r + 2):
                    nc.tensor.matmul(
                        rsp_pk[0:1, :],
                        lhsT=ones8[:, :, 0:1],
                        rhs=pts[m][:, :, r * 512 - bases[m]:
                                   (r + 1) * 512 - bases[m]],
                        start=(m == 0),
                        stop=(m == 2 * r + 1),
                        perf_mode=DR,
                    )
                # broadcast the RAW rowsums (single-lane reciprocal is ~3.3us
                # on DVE; the all-lane bf16 reciprocal after broadcast is
                # ~0.4us), round-tripping through DRAM because SBUF DMA
                # sources cannot have stride-0 partition dims
                ri = ripool.tile([1, 512], BF16, tag="ri", name=f"ri{h}_{r}")
                with nc.allow_low_precision(
                    reason="bf16 rowsum: ~0.4% on attn term vs 2e-2 gate"
                ):
                    nc.vector.tensor_copy(out=ri, in_=rsp_pk[0:1, :])
                rs_slice = rsd[h:h + 1, r * 512:(r + 1) * 512]
                d_wr = nc.sync.dma_start(out=rs_slice, in_=ri)
                rb = rbpool.tile([128, 512], BF16, tag=f"rb{r}",
                                 name=f"rb{h}_{r}")
                for cs in range(2):
                    csl = slice(cs * 256, (cs + 1) * 256)
                    d_rd = nc.sync.dma_start(
                        out=rb[:, csl],
                        in_=_bcast_rows(rsd[h:h + 1, r * 512 + cs * 256:
                                            r * 512 + (cs + 1) * 256]),
                    )
                    _adh(d_rd.ins, d_wr.ins, reason="rowsum bcast RAW via DRAM")
                with nc.allow_low_precision(
                    reason="bf16 1/rowsum: ~0.4% on attn term vs 2e-2 gate"
                ):
                    nc.vector.reciprocal(out=rb, in_=rb)
                rb_h[(h, r)] = rb

            def b_units(h):
                """P@V for head h as a list of (cost_ns, closure) filler
                units: per-m matmul groups + normalized drains."""
                pts = pts_h[h]
                units = []
                state = {}

                def mk_group(half, m):
                    def fn():
                        if m == 0:
                            state[half] = otpool.tile(
                                [128, 1024], F32, tag="ot", name=f"ot{h}_{half}"
                            )
                        ot_ps = state[half]
                        for k in range(2):
                            r = 2 * half + k
                            if m >= 2 * r + 2:
                                continue
                            nc.tensor.matmul(
                                ot_ps[:, k * 512:(k + 1) * 512],
                                lhsT=v8_sb[:, 2 * m:2 * m + 2,
                                           h * DK:(h + 1) * DK],
                                rhs=pts[m][:, :, r * 512 - bases[m]:
                                           (r + 1) * 512 - bases[m]],
                                start=(m == 0),
                                stop=(m == 2 * r + 1),
                                perf_mode=DR,
                            )
                    return fn

                def mk_drain(half):
                    def fn():
                        ot_ps = state[half]
                        for k in range(2):
                            r = 2 * half + k
                            o_sb = osbpool.tile([128, 512], BF16, tag="osb",
                                                name=f"o_sb{h}_{r}")
                            with nc.allow_low_precision(
                                reason="bf16 attn out: ~0.4% vs 2e-2 gate"
                            ):
                                nc.vector.tensor_mul(
                                    out=o_sb, in0=ot_ps[:, k * 512:(k + 1) * 512],
                                    in1=rb_h.pop((h, r)),
                                )
                            for cs in range(2):
                                nc.gpsimd.dma_start(
                                    out=o_t[h, :, r * 512 + cs * 256:
                                            r * 512 + (cs + 1) * 256],
                                    in_=o_sb[:, cs * 256:(cs + 1) * 256],
                                )
                    return fn

                for half in range(2):
                    nm_half = 2 * (2 * half + 1) + 2
                    for m in range(nm_half):
                        n_mm = sum(1 for k in range(2)
                                   if m < 2 * (2 * half + k) + 2)
                        units.append((150 + 213 * n_mm + 135, mk_group(half, m)))
                    units.append((100, mk_drain(half)))
                return units

            # ---------- emission schedule ----------
            # K0/Q0 projections in sc-chunks so the PE starts as soon as the
            # first kp-split DMAs land, and A(0) can begin before xv arrives.
            for sc in range(NS):
                emit_proj_sc(0, sc, wk_t, bk_sb, kt_sb[0])
                emit_proj_sc(0, sc, wq_t, bq_sb, qt_sb[0])

            for h in range(HG):
                emit_pts_alloc(h)
                # filler units woven between A chunks: V projection during
                # head 0, P@V of head h-1 afterwards
                if h == 0:
                    fillers = [(2800, (lambda sc=sc, tt=tt:
                                       v_unit(sc, tt)))
                               for sc in range(NS) for tt in range(2)]
                else:
                    fillers = b_units(h - 1)
                total_fill = sum(c for c, _ in fillers) or 1
                fillers = list(fillers)
                balance = 0.0
                emitted = 0.0
                # ScalarE cost of each A unit, to pace fillers linearly
                a_units = []
                for j in range(NJ):
                    for hl in range(j // 8, 2):
                        qlo = max(hl * 1024, j * 128)
                        a_units.append((j, hl, (1024 * (hl + 1) - qlo) * 0.72
                                        + 260))
                total_a = sum(c for _, _, c in a_units)
                for j, hl, cost in a_units:
                    a_unit(h, j, hl)
                    emitted += cost
                    # rowsums as soon as all P chunks for range r are exp'd
                    if hl == 1 and (j + 1) % 4 == 0:
                        rs_unit(h, (j + 1) // 4 - 1)
                    want = total_fill * emitted / total_a
                    while fillers and balance < want:
                        c, fn = fillers.pop(0)
                        fn()
                        balance += c
                for c, fn in fillers:
                    fn()
                if h + 1 < HG:
                    emit_proj_block(h + 1, wk_t, xk8, bk_sb, kt_sb[h + 1])
                    emit_proj_block(h + 1, wq_t, xq8, bq_sb, qt_sb[h + 1])
            # tail: P@V of the last head
            for c, fn in b_units(HG - 1):
                fn()
    _dedupe_ldweights(nc)
    _split_excess_waits(nc)
    return nc


def _build_layernorm(affine=True):
    """Per-core: residual add + LayerNorm over 1024 rows of [8192, 1024].

    Inputs arrive bf16 and already normalized (softmax applied in the
    attention kernel).  Everything stays bf16 so DVE runs in 2x mode.
    affine=False omits gamma/beta (valid when gamma==1, beta==0)."""
    nc = bass.Bass()
    RPC = (B * S) // NCORES  # 1024 rows per core

    attn = nc.dram_tensor("attn", [RPC, D], BF16, kind="ExternalInput")
    resid = nc.dram_tensor("resid", [RPC, D], BF16, kind="ExternalInput")
    gamma = nc.dram_tensor("gamma", [D], F32, kind="ExternalInput")
    beta = nc.dram_tensor("beta", [D], F32, kind="ExternalInput")
    out = nc.dram_tensor("out", [RPC, D], BF16, kind="ExternalOutput")

    with TileContext(nc) as tc:
        with (
            tc.tile_pool(name="consts", bufs=1) as consts,
            tc.tile_pool(name="work", bufs=3) as work,
            tc.tile_pool(name="stat", bufs=4) as statp,
        ):
            if affine:
                gamma_sb = consts.tile([128, D], BF16)
                beta_sb = consts.tile([128, D], BF16)
                nc.gpsimd.dma_start(
                    out=gamma_sb,
                    in_=bass.AP(tensor=gamma[:].tensor, offset=gamma[:].offset,
                                ap=[[0, 128]] + list(gamma[:].ap)),
                )
                nc.gpsimd.dma_start(
                    out=beta_sb,
                    in_=bass.AP(tensor=beta[:].tensor, offset=beta[:].offset,
                                ap=[[0, 128]] + list(beta[:].ap)),
                )
            eps_sb = consts.tile([128, 1], F32)
            nc.vector.memset(eps_sb, EPS)

            nsub = D // 512  # bn_stats free-dim limit
            NT = RPC // 128
            for t in range(NT):
                ab = work.tile([128, D], BF16, tag="ab", name="ab")
                rb = work.tile([128, D], BF16, tag="rb", name="rb")
                x = work.tile([128, D], BF16, tag="x", name="x")
                nc.sync.dma_start(out=ab, in_=attn[t * 128:(t + 1) * 128, :])
                nc.sync.dma_start(out=rb, in_=resid[t * 128:(t + 1) * 128, :])
                with nc.allow_low_precision(
                    reason="bf16 residual add: ~0.4% vs 2e-2 gate"
                ):
                    nc.vector.tensor_add(out=x, in0=ab, in1=rb)

                stats = statp.tile([128, nsub, 6], F32, tag="stats",
                                   name="stats")
                for sgi in range(nsub):
                    nc.vector.bn_stats(
                        out=stats[:, sgi, :],
                        in_=x[:, sgi * 512:(sgi + 1) * 512],
                    )
                mv = statp.tile([128, 2], F32, tag="mv", name="mv")
                nc.vector.bn_aggr(out=mv, in_=stats)
                rstd = statp.tile([128, 1], F32, tag="rstd", name="rstd")
                nc.scalar.activation(
                    out=rstd,
                    in_=mv[:, 1:2],
                    func=mybir.ActivationFunctionType.Sqrt,
                    bias=eps_sb,
                    scale=1.0,
                )
                nc.vector.reciprocal(out=rstd, in_=rstd)
                xo = work.tile([128, D], BF16, tag="xo", name="xo")
                with nc.allow_low_precision(
                    reason="bf16 LN output: ~0.2% rounding vs 2e-2 gate"
                ):
                    nc.vector.tensor_scalar(
                        out=xo if not affine else x,
                        in0=x,
                        scalar1=mv[:, 0:1],
                        scalar2=rstd,
                        op0=mybir.AluOpType.subtract,
                        op1=mybir.AluOpType.mult,
                    )
                    if affine:
                        nc.vector.tensor_mul(out=x, in0=x, in1=gamma_sb)
                        nc.vector.tensor_add(out=xo, in0=x, in1=beta_sb)
                nc.gpsimd.dma_start(
                    out=out[t * 128:(t + 1) * 128, :], in_=xo
                )
    _split_excess_waits(nc)
    return nc


_CACHE = {}


def _get_programs(affine=True):
    # note: walrus's --enable-ldw-opt=true rejects DoubleRow LDWEIGHTS
    # ("InstLdweights is not compatible with LDW optimization"), so redundant
    # weight loads are removed by _dedupe_ldweights instead.
    if "attn" not in _CACHE:
        _CACHE["attn"] = _build_attention()
    key = ("ln", affine)
    if key not in _CACHE:
        _CACHE[key] = _build_layernorm(affine=affine)
    return _CACHE["attn"], _CACHE[key]


def _prep_x(xb):
    """[S, D] f32 -> [sc, kp, 128, 2, 512] fp8 with
    x8[sc, kp, p, i, s'] = X[sc*512+s', 256*kp+128*i+p]."""
    xT = np.asarray(xb, dtype=np.float32).T  # [D, S]
    arr = xT.reshape(KP, 2, 128, NS, 512).transpose(3, 0, 2, 1, 4)
    return np.ascontiguousarray(arr.astype(NPFP8))


def _prep_w(Wm, g):
    """W[:, g*512:(g+1)*512]*WS -> [128, kp, 2, 512] fp8."""
    ws = np.asarray(Wm, dtype=np.float32)[:, g * HW:(g + 1) * HW] * WS
    arr = ws.reshape(KP, 2, 128, HW).transpose(2, 0, 1, 3)
    return np.ascontiguousarray(arr.astype(NPFP8))


def _run(inputs, trace=False):
    """Returns (output, attn_results, ln_results)."""
    gamma_np = np.asarray(inputs["gamma"], dtype=np.float32)
    beta_np = np.asarray(inputs["beta"], dtype=np.float32)
    affine = not (np.all(gamma_np == 1.0) and np.all(beta_np == 0.0))
    nc_attn, nc_ln = _get_programs(affine=affine)

    q = np.ascontiguousarray(np.asarray(inputs["queries"], dtype=np.float32))
    k = np.ascontiguousarray(np.asarray(inputs["keys"], dtype=np.float32))
    v = np.ascontiguousarray(np.asarray(inputs["values"], dtype=np.float32))
    Wq = np.asarray(inputs["Wq"], dtype=np.float32)
    Wk = np.asarray(inputs["Wk"], dtype=np.float32)
    Wv = np.asarray(inputs["Wv"], dtype=np.float32)
    bq = np.asarray(inputs["bq"], dtype=np.float32)
    bk = np.asarray(inputs["bk"], dtype=np.float32)
    bv = np.asarray(inputs["bv"], dtype=np.float32)

    xt = {}
    for b in range(B):
        xt[("q", b)] = _prep_x(q[b])
        xt[("k", b)] = _prep_x(k[b])
        xt[("v", b)] = _prep_x(v[b])
    wslices = {}
    bslices = {}
    for g in range(2):
        cols = slice(g * HW, (g + 1) * HW)
        wslices[("q", g)] = _prep_w(Wq, g)
        wslices[("k", g)] = _prep_w(Wk, g)
        wslices[("v", g)] = _prep_w(Wv, g)
        bslices[g] = np.ascontiguousarray(np.concatenate([
            (WS * bq[cols]).reshape(HG, 128).T,
            (WS * bk[cols]).reshape(HG, 128).T,
            np.broadcast_to(bv[cols], (128, HW)),
        ], axis=1, dtype=np.float32))

    in_maps = []
    for c in range(NCORES):
        b, g = c // 2, c % 2
        in_maps.append({
            "xq_t": xt[("q", b)],
            "xk_t": xt[("k", b)],
            "xv_t": xt[("v", b)],
            "wq": wslices[("q", g)],
            "wk": wslices[("k", g)],
            "wv": wslices[("v", g)],
            "bqkv": bslices[g],
        })

    res1 = run_bass_kernel_spmd(
        nc_attn, in_maps, core_ids=list(range(NCORES)), trace=trace
    )

    # assemble attention output [B, S, D] bf16 (already normalized in-kernel)
    attn_full = np.empty((B, S, D), dtype=NPBF16)
    for c in range(NCORES):
        b, g = c // 2, c % 2
        ot = res1.results[c]["o_t"]  # [HG, DK, S]
        for i in range(HG):
            attn_full[b, :, (g * HG + i) * DK:(g * HG + i + 1) * DK] = ot[i].T

    attn_flat = attn_full.reshape(B * S, D)
    q_flat = q.reshape(B * S, D).astype(NPBF16)
    RPC = (B * S) // NCORES
    in_maps2 = []
    for c in range(NCORES):
        rows = slice(c * RPC, (c + 1) * RPC)
        in_maps2.append({
            "attn": np.ascontiguousarray(attn_flat[rows]),
            "resid": np.ascontiguousarray(q_flat[rows]),
            "gamma": gamma_np,
            "beta": beta_np,
        })
    res2 = run_bass_kernel_spmd(
        nc_ln, in_maps2, core_ids=list(range(NCORES)), trace=trace
    )
    out = np.concatenate(
        [res2.results[c]["out"].astype(np.float32) for c in range(NCORES)],
        axis=0,
    ).reshape(B, S, D)
    return out, res1, res2


def kernel(**inputs):
    out, _, _ = _run(inputs, trace=False)
    return out


# revision 14
# speedup vs baseline: 1.1670x; 1.1490x over previous
"""Trainium2 Bass kernel for causal MultiHeadAttention + residual + LayerNorm.

Problem shapes (hardcoded):
  B=4, S=2048, D_MODEL=1024, H=8 heads, d_k=128.
  out = LayerNorm(queries + MHA(queries, keys, values))

Sharding (8 cores):
  Launch 1 (attention): core c <-> (batch b = c//2, head group g = c%2 -> heads
  4g..4g+3).  Q/K/V weights column-sharded by head group.
  All big matmuls except QK^T run in fp8e4 DoubleRow perf mode (256-deep
  contraction per pass, 2x PE throughput); QK^T stays bf16 (contraction is
  d_k=128, DoubleRow cannot apply).

  Schedule: the PE stream is software-pipelined so the ScalarE-bound exp
  windows of head h are filled with PE work: V projection (head 0's window),
  P@V of head h-1, rowsum matmuls of head h (emitted as soon as their P
  chunks are exp'd), and K/Q projections of head h+1 at the window tail.
  Rowsums are inverted (DVE reciprocal from PSUM) and partition-broadcast
  via SBUF->SBUF DMA, so O^T is normalized during its PSUM->SBUF drain and
  the attention output DMA'd to DRAM is the final softmax(QK^T)V.

  Launch 2 (layernorm): row-sharded, 1024 rows of the flattened [8192,1024]
  residual per core; pure bf16 add + bn_stats + normalize.
"""

import sys

import numpy as np

for _p in ("/opt/trn_rl_repo", "/opt/pypackages"):
    if _p not in sys.path:
        sys.path.append(_p)

import ml_dtypes  # noqa: E402

import concourse.bass as bass  # noqa: E402
import concourse.mybir as mybir  # noqa: E402
from concourse.tile import TileContext  # noqa: E402
from concourse.tile import add_dep_helper as _adh  # noqa: E402
from concourse.bass_utils import run_bass_kernel_spmd  # noqa: E402
from concourse.masks import make_lower_triangular  # noqa: E402

B = 4
S = 2048
D = 1024
H = 8
DK = 128
HG = 4  # heads per core
NCORES = 8
WS = 32.0  # host-side weight scale so fp8 sees ~N(0,1) values
SCALE = 1.0 / np.sqrt(np.float32(DK))
C_SHIFT = 2.0  # exp(s - C): keeps fp8 P below overflow (TRN e4m3 max 240)
NEG_INF = -1e9
EPS = 1e-6

BF16 = mybir.dt.bfloat16
F32 = mybir.dt.float32
FP8 = mybir.dt.float8e4
NPBF16 = ml_dtypes.bfloat16
NPFP8 = ml_dtypes.float8_e4m3  # IEEE e4m3 (max 240) == TRN FP8_EXP4
DR = mybir.MatmulPerfMode.DoubleRow

KP = D // 256   # 4 contraction pair-chunks (256 rows each)
NS = S // 512   # 4 query ranges of 512
NJ = S // 128   # 16 key chunks of 128
NM = NJ // 2    # 8 key pair-chunks of 256
HW = HG * DK    # 512 columns per head group


def _bcast_rows(ap, n=128):
    """Broadcast a row across n partitions (stride-0 partition dim)."""
    return bass.AP(tensor=ap.tensor, offset=ap.offset, ap=[[0, n]] + list(ap.ap)[1:])


def _dedupe_ldweights(nc):
    """Remove InstLdweights that reload the exact weights already resident in
    the PE array (same AP/perf_mode/tile_position as the previous LDW on the
    PE stream, nothing reloaded between).  All stationary tiles in this
    kernel are write-once, so AP identity implies content identity.  LDWs
    carry no sem updates here, so deletion cannot break downstream waits;
    LDWs that carry waits are kept.  Each deleted LDW saves ~100ns of PE
    sequencer dispatch."""
    n_del = 0
    for f in nc.m.functions:
        for bb in f.blocks:
            il = bb.instructions
            out = []
            pk = None
            changed = False
            for ins in il:
                tname = type(ins).__name__
                if tname == "InstLdweights":
                    si = ins.sync_info
                    has_sync = si is not None and (
                        len(si.on_wait) > 0 or len(si.on_update) > 0
                    )
                    key = (
                        str(ins.ins[0]),
                        str(ins.perf_mode),
                        str(ins.tile_position),
                        str(ins.is_transpose),
                    )
                    if key == pk and not has_sync:
                        n_del += 1
                        changed = True
                        continue
                    pk = key
                elif tname == "InstMatmult" and getattr(ins, "is_transpose", None):
                    pk = None  # transpose clobbers the loaded weights
                out.append(ins)
            if changed:
                il[:] = out
    return n_del


def _split_excess_waits(nc):
    """Workaround for this walrus build: engine (TPB) instructions accept at
    most one sync-wait command (EventSemaphore: two), but Tile attaches one
    wait per dependency.  Move excess waits onto same-engine NOPs inserted
    immediately before the over-limit instruction."""
    n_new = 0
    for f in nc.m.functions:
        for bb in f.blocks:
            il = bb.instructions
            out = []
            changed = False
            for ins in il:
                si = ins.sync_info
                tname = type(ins).__name__
                if si is not None:
                    cap = 2 if tname == "InstEventSemaphore" else 1
                    waits = list(si.on_wait)
                    if len(waits) > cap:
                        for w in waits[cap:]:
                            nop = mybir.InstNoOp(
                                name=f"I-wsplit-{n_new}",
                                sync_info=mybir.SyncInfo(
                                    on_wait=[w], on_update=[]
                                ),
                                bass_nofuse=True,
                                engine=ins.engine,
                            )
                            n_new += 1
                            out.append(nop)
                        si.on_wait = waits[:cap]
                        changed = True
                out.append(ins)
            if changed:
                il[:] = out
    return n_new


def _build_attention():
    """Per-core attention program: 4 heads of one batch, fp8 DoubleRow."""
    nc = bass.Bass()

    # activations pre-chunked on host: [sc, kp, 128, 2, 512] fp8 so each
    # (sc, kp) DMA piece is fully contiguous (fast descriptor generation)
    xq_t = nc.dram_tensor("xq_t", [NS, KP, 128, 2, 512], FP8, kind="ExternalInput")
    xk_t = nc.dram_tensor("xk_t", [NS, KP, 128, 2, 512], FP8, kind="ExternalInput")
    xv_t = nc.dram_tensor("xv_t", [NS, KP, 128, 2, 512], FP8, kind="ExternalInput")
    # weights pre-permuted+scaled on host: [128, kp, 2, 512] fp8
    wq = nc.dram_tensor("wq", [128, KP, 2, HW], FP8, kind="ExternalInput")
    wk = nc.dram_tensor("wk", [128, KP, 2, HW], FP8, kind="ExternalInput")
    wv = nc.dram_tensor("wv", [128, KP, 2, HW], FP8, kind="ExternalInput")
    # biases packed [bq32 | bk32 | bv_bcast]: [128, HG+HG+HW] f32
    bqkv = nc.dram_tensor("bqkv", [128, 2 * HG + HW], F32, kind="ExternalInput")
    # per-head NORMALIZED attention output O^T (softmax applied in-kernel)
    o_t = nc.dram_tensor("o_t", [HG, DK, S], BF16, kind="ExternalOutput")
    # DRAM scratch for the 1/rowsum partition-broadcast round trip (SBUF
    # sources cannot have stride-0 partition dims in DMA APs)
    rsd = nc.dram_tensor("rsd", [HG, S], BF16, kind="Internal")

    with TileContext(nc) as tc:
        from contextlib import ExitStack

        with ExitStack() as ctx:
            consts = ctx.enter_context(tc.tile_pool(name="consts", bufs=1))
            xpool = ctx.enter_context(tc.tile_pool(name="x", bufs=1))
            wpool = ctx.enter_context(tc.tile_pool(name="w", bufs=1))
            proj_out = ctx.enter_context(tc.tile_pool(name="proj_out", bufs=1))
            ptpool = ctx.enter_context(tc.tile_pool(name="pt", bufs=2))
            osbpool = ctx.enter_context(tc.tile_pool(name="osb", bufs=4))
            ripool = ctx.enter_context(tc.tile_pool(name="ri", bufs=4))
            rbpool = ctx.enter_context(tc.tile_pool(name="rb", bufs=2))
            stpool = ctx.enter_context(
                tc.tile_pool(name="st", bufs=2, space="PSUM")
            )
            rspool = ctx.enter_context(
                tc.tile_pool(name="rsp", bufs=2, space="PSUM")
            )
            otpool = ctx.enter_context(
                tc.tile_pool(name="ot", bufs=1, space="PSUM")
            )

            # --- constants ---
            tril = consts.tile([128, 128], F32)  # additive: -1e9 where k > q
            make_lower_triangular(nc, tril, val=NEG_INF, diag=False)
            # pair-dim stride must be 16B-aligned for dual-fp8 LDWEIGHTS
            ones8 = consts.tile([128, 2, 16], FP8)
            nc.vector.memset(ones8, 1.0)
            negc_sb = consts.tile([128, 1], F32)
            nc.vector.memset(negc_sb, -float(C_SHIFT))
            zero_sb = consts.tile([128, 1], F32)
            nc.vector.memset(zero_sb, 0.0)
            b_sb = consts.tile([128, 2 * HG + HW], F32)
            # bias DMA split so the first K drain doesn't wait on one big DMA
            nc.scalar.dma_start(out=b_sb[:, 0:2 * HG], in_=bqkv[:, 0:2 * HG])
            nc.scalar.dma_start(
                out=b_sb[:, 2 * HG:2 * HG + 256], in_=bqkv[:, 2 * HG:2 * HG + 256]
            )
            nc.scalar.dma_start(
                out=b_sb[:, 2 * HG + 256:], in_=bqkv[:, 2 * HG + 256:]
            )
            bq_sb = b_sb[:, 0:HG]
            bk_sb = b_sb[:, HG:2 * HG]
            bv_sb = b_sb[:, 2 * HG:]

            # --- SBUF tiles for activations / projections ---
            xq8 = xpool.tile([128, NS, KP, 2, 512], FP8, tag="xq", name="xq")
            xk8 = xpool.tile([128, NS, KP, 2, 512], FP8, tag="xk", name="xk")
            xv8 = xpool.tile([128, NS, KP, 2, 512], FP8, tag="xv", name="xv")
            wq_t = wpool.tile([128, KP, 2, HW], FP8, tag="wq_t", name="wq_t")
            wk_t = wpool.tile([128, KP, 2, HW], FP8, tag="wk_t", name="wk_t")
            wv_t = wpool.tile([128, KP, 2, HW], FP8, tag="wv_t", name="wv_t")
            qt_sb = [proj_out.tile([128, S], BF16, tag=f"qt{h}", name=f"qt{h}")
                     for h in range(HG)]
            kt_sb = [proj_out.tile([128, S], BF16, tag=f"kt{h}", name=f"kt{h}")
                     for h in range(HG)]
            v8_sb = proj_out.tile([128, NJ, HW], FP8, tag="v8", name="v8")

            # --- input DMAs, kp-split for queue parallelism, spread across
            # issue engines so descriptor generation doesn't serialize ---
            for kp in range(KP):
                nc.sync.dma_start(out=wk_t[:, kp], in_=wk[:, kp])
            for sc in range(NS):
                for kp in range(KP):
                    nc.sync.dma_start(out=xk8[:, sc, kp], in_=xk_t[sc, kp])
            for kp in range(KP):
                nc.scalar.dma_start(out=wq_t[:, kp], in_=wq[:, kp])
            for sc in range(NS):
                for kp in range(KP):
                    nc.gpsimd.dma_start(out=xq8[:, sc, kp], in_=xq_t[sc, kp])
            for kp in range(KP):
                nc.gpsimd.dma_start(out=wv_t[:, kp], in_=wv[:, kp])
            for sc in range(NS):
                for kp in range(KP):
                    nc.gpsimd.dma_start(out=xv8[:, sc, kp], in_=xv_t[sc, kp])

            # ---------- emission helpers ----------
            bases = [512 * (m // 2) for m in range(NM)]
            pts_h = {}
            rb_h = {}   # (h, r) -> broadcast 1/rowsum tile [128, 512] bf16

            def emit_proj_sc(h, sc, w_t, b_col, out_sb):
                """One 512-query chunk of a K^T/Q^T projection (kp inner, so
                it can start as soon as that sc's activations land)."""
                ps = rspool.tile([128, 512], F32, tag="rsp", name=f"pj{h}{sc}")
                for kp in range(KP):
                    nc.tensor.matmul(
                        ps,
                        lhsT=w_t[:, kp, :, h * DK:(h + 1) * DK],
                        rhs=xk8[:, sc, kp] if w_t is wk_t else xq8[:, sc, kp],
                        start=(kp == 0),
                        stop=(kp == KP - 1),
                        perf_mode=DR,
                    )
                nc.vector.tensor_scalar_add(
                    out=out_sb[:, sc * 512:(sc + 1) * 512],
                    in0=ps,
                    scalar1=b_col[:, h:h + 1],
                )

            def emit_proj_block(h, w_t, x_t8, b_col, out_sb):
                """K^T/Q^T projection for one head, kp outer (4 LDWs)."""
                pss = [stpool.tile([128, 1024], F32, tag="st", name=f"ps{h}")
                       for _ in range(2)]
                for kp in range(KP):
                    for sc in range(4):
                        nc.tensor.matmul(
                            pss[sc // 2][:, (sc % 2) * 512:(sc % 2 + 1) * 512],
                            lhsT=w_t[:, kp, :, h * DK:(h + 1) * DK],
                            rhs=x_t8[:, sc, kp],
                            start=(kp == 0),
                            stop=(kp == KP - 1),
                            perf_mode=DR,
                        )
                for st2 in range(2):
                    nc.vector.tensor_scalar_add(
                        out=out_sb[:, st2 * 1024:(st2 + 1) * 1024],
                        in0=pss[st2],
                        scalar1=b_col[:, h:h + 1],
                    )

            def v_unit(sc, tt):
                """V projection chunk: V[sc queries, cols] for one 256-row
                slab pair; fp8 output at true scale."""
                ps = stpool.tile([128, 1024], F32, tag="st", name="psv")
                for sbl in range(2):
                    sbl2 = 2 * tt + sbl
                    for kp in range(KP):
                        nc.tensor.matmul(
                            ps[:, sbl * 512:(sbl + 1) * 512],
                            lhsT=xv8[:, sc, kp, :,
                                     sbl2 * 128:(sbl2 + 1) * 128],
                            rhs=wv_t[:, kp],
                            start=(kp == 0),
                            stop=(kp == KP - 1),
                            perf_mode=DR,
                        )
                for sbl in range(2):
                    sb = 4 * sc + 2 * tt + sbl
                    nc.vector.scalar_tensor_tensor(
                        out=v8_sb[:, sb, :],
                        in0=ps[:, sbl * 512:(sbl + 1) * 512],
                        scalar=1.0 / WS,
                        in1=bv_sb,
                        op0=mybir.AluOpType.mult,
                        op1=mybir.AluOpType.add,
                    )

            def emit_pts_alloc(h):
                pts = []
                for m in range(NM):
                    pt = ptpool.tile([128, 2, S - bases[m]], FP8,
                                     tag=f"ptp{m}", name=f"pt{h}_{m}")
                    pts.append(pt)
                pts_h[h] = pts
                # zero the causally-invalid diagonal blocks so DoubleRow
                # P@V / rowsum matmuls can run unmasked over full pairs
                # (on GpSimd to keep the DVE queue clear)
                for r in range(NS):
                    nc.gpsimd.memset(pts[2 * r][:, 1, 0:128], 0.0)
                    nc.gpsimd.memset(pts[2 * r + 1][:, :, 0:384], 0.0)

            def a_unit(h, j, hl):
                """Scores for key chunk j, query cols [hl*1024,(hl+1)*1024),
                plus the exp into fp8 P^T."""
                pts = pts_h[h]
                m = j // 2
                jq = j * 128
                base = bases[m]
                qlo = max(hl * 1024, jq)
                a = qlo - hl * 1024
                r0 = j // 4
                st = stpool.tile([128, 1024], F32, tag="st", name="st")
                for r in range(max(2 * hl, r0), 2 * hl + 2):
                    rqlo = max(r * 512, jq)
                    ra = rqlo - hl * 1024
                    nc.tensor.matmul(
                        st[:, ra:(r + 1) * 512 - hl * 1024],
                        lhsT=kt_sb[h][:, jq:jq + 128],
                        rhs=qt_sb[h][:, rqlo:(r + 1) * 512],
                        start=True,
                        stop=True,
                    )
                if qlo == jq:
                    nc.vector.tensor_add(
                        out=st[:, a:a + 128],
                        in0=st[:, a:a + 128],
                        in1=tril,
                    )
                nc.scalar.activation(
                    out=pts[m][:, j % 2, qlo - base:(hl + 1) * 1024 - base],
                    in_=st[:, a:1024],
                    func=mybir.ActivationFunctionType.Exp,
                    scale=float(SCALE / (WS * WS)),
                    bias=negc_sb,
                )

            def rs_unit(h, r):
                """Rowsums for query range r (fp8 DoubleRow ones-matmul over
                all P chunks), then 1/rowsum broadcast across partitions so
                the O^T drain can normalize in place."""
                pts = pts_h[h]
                rsp_pk = rspool.tile([128, 512], F32, tag="rsp",
                                     name=f"rsp{h}_{r}")
                for m in range(2 * r + 2):
                    nc.tensor.matmul(
                        rsp_pk[0:1, :],
                        lhsT=ones8[:, :, 0:1],
                        rhs=pts[m][:, :, r * 512 - bases[m]:
                                   (r + 1) * 512 - bases[m]],
                        start=(m == 0),
                        stop=(m == 2 * r + 1),
                        perf_mode=DR,
                    )
                # 1/rowsum = Exp(-Ln(rowsum)) on ScalarE: DVE's reciprocal is
                # a ~3.3us multipass op and was clogging the vector queue;
                # two ScalarE table ACTs cost ~1.4us and read the PSUM row
                # directly.  Then partition-broadcast via a DRAM round trip
                # (SBUF DMA sources cannot have stride-0 partition dims).
                lntmp = ripool.tile([1, 512], F32, tag="ln", name=f"ln{h}_{r}")
                nc.scalar.activation(
                    out=lntmp,
                    in_=rsp_pk[0:1, :],
                    func=mybir.ActivationFunctionType.Ln,
                    bias=zero_sb[0:1, :],
                )
                ri = ripool.tile([1, 512], BF16, tag="ri", name=f"ri{h}_{r}")
                with nc.allow_low_precision(
                    reason="bf16 1/rowsum: ~0.4% on attn term vs 2e-2 gate"
                ):
                    nc.scalar.activation(
                        out=ri,
                        in_=lntmp,
                        func=mybir.ActivationFunctionType.Exp,
                        scale=-1.0,
                        bias=zero_sb[0:1, :],
                    )
                rs_slice = rsd[h:h + 1, r * 512:(r + 1) * 512]
                d_wr = nc.sync.dma_start(out=rs_slice, in_=ri)
                rb = rbpool.tile([128, 512], BF16, tag=f"rb{r}",
                                 name=f"rb{h}_{r}")
                for cs in range(2):
                    d_rd = nc.sync.dma_start(
                        out=rb[:, cs * 256:(cs + 1) * 256],
                        in_=_bcast_rows(rsd[h:h + 1, r * 512 + cs * 256:
                                            r * 512 + (cs + 1) * 256]),
                    )
                    _adh(d_rd.ins, d_wr.ins, reason="rowsum bcast RAW via DRAM")
                rb_h[(h, r)] = rb

            def b_units(h):
                """P@V for head h as a list of (cost_ns, closure) filler
                units: per-m matmul groups + normalized drains."""
                pts = pts_h[h]
                units = []
                state = {}

                def mk_group(half, m):
                    def fn():
                        if m == 0:
                            state[half] = otpool.tile(
                                [128, 1024], F32, tag="ot", name=f"ot{h}_{half}"
                            )
                        ot_ps = state[half]
                        for k in range(2):
                            r = 2 * half + k
                            if m >= 2 * r + 2:
                                continue
                            nc.tensor.matmul(
                                ot_ps[:, k * 512:(k + 1) * 512],
                                lhsT=v8_sb[:, 2 * m:2 * m + 2,
                                           h * DK:(h + 1) * DK],
                                rhs=pts[m][:, :, r * 512 - bases[m]:
                                           (r + 1) * 512 - bases[m]],
                                start=(m == 0),
                                stop=(m == 2 * r + 1),
                                perf_mode=DR,
                            )
                    return fn

                def mk_drain(half):
                    def fn():
                        ot_ps = state[half]
                        for k in range(2):
                            r = 2 * half + k
                            o_sb = osbpool.tile([128, 512], BF16, tag="osb",
                                                name=f"o_sb{h}_{r}")
                            with nc.allow_low_precision(
                                reason="bf16 attn out: ~0.4% vs 2e-2 gate"
                            ):
                                nc.vector.tensor_mul(
                                    out=o_sb, in0=ot_ps[:, k * 512:(k + 1) * 512],
                                    in1=rb_h.pop((h, r)),
                                )
                            for cs in range(2):
                                nc.gpsimd.dma_start(
                                    out=o_t[h, :, r * 512 + cs * 256:
                                            r * 512 + (cs + 1) * 256],
                                    in_=o_sb[:, cs * 256:(cs + 1) * 256],
                                )
                    return fn

                for half in range(2):
                    nm_half = 2 * (2 * half + 1) + 2
                    for m in range(nm_half):
                        n_mm = sum(1 for k in range(2)
                                   if m < 2 * (2 * half + k) + 2)
                        units.append((150 + 213 * n_mm + 135, mk_group(half, m)))
                    units.append((100, mk_drain(half)))
                return units

            # ---------- emission schedule ----------
            # K0/Q0 projections in sc-chunks so the PE starts as soon as the
            # first kp-split DMAs land, and A(0) can begin before xv arrives.
            for sc in range(NS):
                emit_proj_sc(0, sc, wk_t, bk_sb, kt_sb[0])
                emit_proj_sc(0, sc, wq_t, bq_sb, qt_sb[0])

            for h in range(HG):
                emit_pts_alloc(h)
                # filler units woven between A chunks: V projection during
                # head 0, P@V of head h-1 afterwards
                if h == 0:
                    fillers = [(2800, (lambda sc=sc, tt=tt:
                                       v_unit(sc, tt)))
                               for sc in range(NS) for tt in range(2)]
                else:
                    fillers = b_units(h - 1)
                total_fill = sum(c for c, _ in fillers) or 1
                fillers = list(fillers)
                balance = 0.0
                emitted = 0.0
                # ScalarE cost of each A unit, to pace fillers linearly
                a_units = []
                for j in range(NJ):
                    for hl in range(j // 8, 2):
                        qlo = max(hl * 1024, j * 128)
                        a_units.append((j, hl, (1024 * (hl + 1) - qlo) * 0.72
                                        + 260))
                total_a = sum(c for _, _, c in a_units)
                for j, hl, cost in a_units:
                    a_unit(h, j, hl)
                    emitted += cost
                    # rowsums as soon as all P chunks for range r are exp'd
                    if hl == 1 and (j + 1) % 4 == 0:
                        rs_unit(h, (j + 1) // 4 - 1)
                    want = total_fill * emitted / total_a
                    while fillers and balance < want:
                        c, fn = fillers.pop(0)
                        fn()
                        balance += c
                for c, fn in fillers:
                    fn()
                if h + 1 < HG:
                    emit_proj_block(h + 1, wk_t, xk8, bk_sb, kt_sb[h + 1])
                    emit_proj_block(h + 1, wq_t, xq8, bq_sb, qt_sb[h + 1])
            # tail: P@V of the last head
            for c, fn in b_units(HG - 1):
                fn()
    _dedupe_ldweights(nc)
    _split_excess_waits(nc)
    return nc


def _build_layernorm(affine=True):
    """Per-core: residual add + LayerNorm over 1024 rows of [8192, 1024].

    Inputs arrive bf16 and already normalized (softmax applied in the
    attention kernel).  Everything stays bf16 so DVE runs in 2x mode.
    affine=False omits gamma/beta (valid when gamma==1, beta==0)."""
    nc = bass.Bass()
    RPC = (B * S) // NCORES  # 1024 rows per core

    attn = nc.dram_tensor("attn", [RPC, D], BF16, kind="ExternalInput")
    resid = nc.dram_tensor("resid", [RPC, D], BF16, kind="ExternalInput")
    gamma = nc.dram_tensor("gamma", [D], F32, kind="ExternalInput")
    beta = nc.dram_tensor("beta", [D], F32, kind="ExternalInput")
    out = nc.dram_tensor("out", [RPC, D], BF16, kind="ExternalOutput")

    with TileContext(nc) as tc:
        with (
            tc.tile_pool(name="consts", bufs=1) as consts,
            tc.tile_pool(name="work", bufs=3) as work,
            tc.tile_pool(name="stat", bufs=4) as statp,
        ):
            if affine:
                gamma_sb = consts.tile([128, D], BF16)
                beta_sb = consts.tile([128, D], BF16)
                nc.gpsimd.dma_start(
                    out=gamma_sb,
                    in_=bass.AP(tensor=gamma[:].tensor, offset=gamma[:].offset,
                                ap=[[0, 128]] + list(gamma[:].ap)),
                )
                nc.gpsimd.dma_start(
                    out=beta_sb,
                    in_=bass.AP(tensor=beta[:].tensor, offset=beta[:].offset,
                                ap=[[0, 128]] + list(beta[:].ap)),
                )
            eps_sb = consts.tile([128, 1], F32)
            nc.vector.memset(eps_sb, EPS)

            nsub = D // 512  # bn_stats free-dim limit
            NT = RPC // 128
            for t in range(NT):
                ab = work.tile([128, D], BF16, tag="ab", name="ab")
                rb = work.tile([128, D], BF16, tag="rb", name="rb")
                x = work.tile([128, D], BF16, tag="x", name="x")
                nc.sync.dma_start(out=ab, in_=attn[t * 128:(t + 1) * 128, :])
                nc.sync.dma_start(out=rb, in_=resid[t * 128:(t + 1) * 128, :])
                with nc.allow_low_precision(
                    reason="bf16 residual add: ~0.4% vs 2e-2 gate"
                ):
                    nc.vector.tensor_add(out=x, in0=ab, in1=rb)

                stats = statp.tile([128, nsub, 6], F32, tag="stats",
                                   name="stats")
                for sgi in range(nsub):
                    nc.vector.bn_stats(
                        out=stats[:, sgi, :],
                        in_=x[:, sgi * 512:(sgi + 1) * 512],
                    )
                mv = statp.tile([128, 2], F32, tag="mv", name="mv")
                nc.vector.bn_aggr(out=mv, in_=stats)
                rstd = statp.tile([128, 1], F32, tag="rstd", name="rstd")
                nc.scalar.activation(
                    out=rstd,
                    in_=mv[:, 1:2],
                    func=mybir.ActivationFunctionType.Sqrt,
                    bias=eps_sb,
                    scale=1.0,
                )
                nc.vector.reciprocal(out=rstd, in_=rstd)
                xo = work.tile([128, D], BF16, tag="xo", name="xo")
                with nc.allow_low_precision(
                    reason="bf16 LN output: ~0.2% rounding vs 2e-2 gate"
                ):
                    nc.vector.tensor_scalar(
                        out=xo if not affine else x,
                        in0=x,
                        scalar1=mv[:, 0:1],
                        scalar2=rstd,
                        op0=mybir.AluOpType.subtract,
                        op1=mybir.AluOpType.mult,
                    )
                    if affine:
                        nc.vector.tensor_mul(out=x, in0=x, in1=gamma_sb)
                        nc.vector.tensor_add(out=xo, in0=x, in1=beta_sb)
                nc.gpsimd.dma_start(
                    out=out[t * 128:(t + 1) * 128, :], in_=xo
                )
    _split_excess_waits(nc)
    return nc


_CACHE = {}


def _get_programs(affine=True):
    # note: walrus's --enable-ldw-opt=true rejects DoubleRow LDWEIGHTS
    # ("InstLdweights is not compatible with LDW optimization"), so redundant
    # weight loads are removed by _dedupe_ldweights instead.
    if "attn" not in _CACHE:
        _CACHE["attn"] = _build_attention()
    key = ("ln", affine)
    if key not in _CACHE:
        _CACHE[key] = _build_layernorm(affine=affine)
    return _CACHE["attn"], _CACHE[key]


def _prep_x(xb):
    """[S, D] f32 -> [sc, kp, 128, 2, 512] fp8 with
    x8[sc, kp, p, i, s'] = X[sc*512+s', 256*kp+128*i+p]."""
    xT = np.asarray(xb, dtype=np.float32).T  # [D, S]
    arr = xT.reshape(KP, 2, 128, NS, 512).transpose(3, 0, 2, 1, 4)
    return np.ascontiguousarray(arr.astype(NPFP8))


def _prep_w(Wm, g):
    """W[:, g*512:(g+1)*512]*WS -> [128, kp, 2, 512] fp8."""
    ws = np.asarray(Wm, dtype=np.float32)[:, g * HW:(g + 1) * HW] * WS
    arr = ws.reshape(KP, 2, 128, HW).transpose(2, 0, 1, 3)
    return np.ascontiguousarray(arr.astype(NPFP8))


def _run(inputs, trace=False):
    """Returns (output, attn_results, ln_results)."""
    gamma_np = np.asarray(inputs["gamma"], dtype=np.float32)
    beta_np = np.asarray(inputs["beta"], dtype=np.float32)
    affine = not (np.all(gamma_np == 1.0) and np.all(beta_np == 0.0))
    nc_attn, nc_ln = _get_programs(affine=affine)

    q = np.ascontiguousarray(np.asarray(inputs["queries"], dtype=np.float32))
    k = np.ascontiguousarray(np.asarray(inputs["keys"], dtype=np.float32))
    v = np.ascontiguousarray(np.asarray(inputs["values"], dtype=np.float32))
    Wq = np.asarray(inputs["Wq"], dtype=np.float32)
    Wk = np.asarray(inputs["Wk"], dtype=np.float32)
    Wv = np.asarray(inputs["Wv"], dtype=np.float32)
    bq = np.asarray(inputs["bq"], dtype=np.float32)
    bk = np.asarray(inputs["bk"], dtype=np.float32)
    bv = np.asarray(inputs["bv"], dtype=np.float32)

    xt = {}
    for b in range(B):
        xt[("q", b)] = _prep_x(q[b])
        xt[("k", b)] = _prep_x(k[b])
        xt[("v", b)] = _prep_x(v[b])
    wslices = {}
    bslices = {}
    for g in range(2):
        cols = slice(g * HW, (g + 1) * HW)
        wslices[("q", g)] = _prep_w(Wq, g)
        wslices[("k", g)] = _prep_w(Wk, g)
        wslices[("v", g)] = _prep_w(Wv, g)
        bslices[g] = np.ascontiguousarray(np.concatenate([
            (WS * bq[cols]).reshape(HG, 128).T,
            (WS * bk[cols]).reshape(HG, 128).T,
            np.broadcast_to(bv[cols], (128, HW)),
        ], axis=1, dtype=np.float32))

    in_maps = []
    for c in range(NCORES):
        b, g = c // 2, c % 2
        in_maps.append({
            "xq_t": xt[("q", b)],
            "xk_t": xt[("k", b)],
            "xv_t": xt[("v", b)],
            "wq": wslices[("q", g)],
            "wk": wslices[("k", g)],
            "wv": wslices[("v", g)],
            "bqkv": bslices[g],
        })

    res1 = run_bass_kernel_spmd(
        nc_attn, in_maps, core_ids=list(range(NCORES)), trace=trace
    )

    # assemble attention output [B, S, D] bf16 (already normalized in-kernel)
    attn_full = np.empty((B, S, D), dtype=NPBF16)
    for c in range(NCORES):
        b, g = c // 2, c % 2
        ot = res1.results[c]["o_t"]  # [HG, DK, S]
        for i in range(HG):
            attn_full[b, :, (g * HG + i) * DK:(g * HG + i + 1) * DK] = ot[i].T

    attn_flat = attn_full.reshape(B * S, D)
    q_flat = q.reshape(B * S, D).astype(NPBF16)
    RPC = (B * S) // NCORES
    in_maps2 = []
    for c in range(NCORES):
        rows = slice(c * RPC, (c + 1) * RPC)
        in_maps2.append({
            "attn": np.ascontiguousarray(attn_flat[rows]),
            "resid": np.ascontiguousarray(q_flat[rows]),
            "gamma": gamma_np,
            "beta": beta_np,
        })
    res2 = run_bass_kernel_spmd(
        nc_ln, in_maps2, core_ids=list(range(NCORES)), trace=trace
    )
    out = np.concatenate(
        [res2.results[c]["out"].astype(np.float32) for c in range(NCORES)],
        axis=0,
    ).reshape(B, S, D)
    return out, res1, res2


def kernel(**inputs):
    out, _, _ = _run(inputs, trace=False)
    return out


# revision 29
# speedup vs baseline: 1.1813x; 1.0122x over previous
"""Trainium2 Bass kernel for causal MultiHeadAttention + residual + LayerNorm.

Problem shapes (hardcoded):
  B=4, S=2048, D_MODEL=1024, H=8 heads, d_k=128.
  out = LayerNorm(queries + MHA(queries, keys, values))

Sharding (8 cores):
  Launch 1 (attention): core c <-> (batch b = c//2, head group g = c%2 -> heads
  4g..4g+3).  Q/K/V weights column-sharded by head group.
  All big matmuls except QK^T run in fp8e4 DoubleRow perf mode (256-deep
  contraction per pass, 2x PE throughput); QK^T stays bf16 (contraction is
  d_k=128, DoubleRow cannot apply).

  Schedule: the PE stream is software-pipelined so the ScalarE-bound exp
  windows of head h are filled with PE work: V projection (head 0's window),
  P@V of head h-1, rowsum matmuls of head h (emitted as soon as their P
  chunks are exp'd), and K/Q projections of head h+1 at the window tail.
  Rowsums are inverted (DVE reciprocal from PSUM) and partition-broadcast
  via SBUF->SBUF DMA, so O^T is normalized during its PSUM->SBUF drain and
  the attention output DMA'd to DRAM is the final softmax(QK^T)V.

  Launch 2 (layernorm): row-sharded, 1024 rows of the flattened [8192,1024]
  residual per core; pure bf16 add + bn_stats + normalize.
"""

import sys

import numpy as np

for _p in ("/opt/trn_rl_repo", "/opt/pypackages"):
    if _p not in sys.path:
        sys.path.append(_p)

import ml_dtypes  # noqa: E402

import concourse.bass as bass  # noqa: E402
import concourse.mybir as mybir  # noqa: E402
from concourse.tile import TileContext  # noqa: E402
from concourse.tile import add_dep_helper as _adh  # noqa: E402
from concourse.bass_utils import run_bass_kernel_spmd  # noqa: E402
from concourse.masks import make_lower_triangular  # noqa: E402

B = 4
S = 2048
D = 1024
H = 8
DK = 128
HG = 4  # heads per core
NCORES = 8
WS = 32.0  # host-side weight scale so fp8 sees ~N(0,1) values
SCALE = 1.0 / np.sqrt(np.float32(DK))
C_SHIFT = 2.0  # exp(s - C): keeps fp8 P below overflow (TRN e4m3 max 240)
NEG_INF = -1e9
EPS = 1e-6

BF16 = mybir.dt.bfloat16
F32 = mybir.dt.float32
FP8 = mybir.dt.float8e4
NPBF16 = ml_dtypes.bfloat16
NPFP8 = ml_dtypes.float8_e4m3  # IEEE e4m3 (max 240) == TRN FP8_EXP4
DR = mybir.MatmulPerfMode.DoubleRow

KP = D // 256   # 4 contraction pair-chunks (256 rows each)
NS = S // 512   # 4 query ranges of 512
NJ = S // 128   # 16 key chunks of 128
NM = NJ // 2    # 8 key pair-chunks of 256
HW = HG * DK    # 512 columns per head group


def _bcast_rows(ap, n=128):
    """Broadcast a row across n partitions (stride-0 partition dim)."""
    return bass.AP(tensor=ap.tensor, offset=ap.offset, ap=[[0, n]] + list(ap.ap)[1:])


def _dedupe_ldweights(nc):
    """Remove InstLdweights that reload the exact weights already resident in
    the PE array (same AP/perf_mode/tile_position as the previous LDW on the
    PE stream, nothing reloaded between).  All stationary tiles in this
    kernel are write-once, so AP identity implies content identity.  LDWs
    carry no sem updates here, so deletion cannot break downstream waits;
    LDWs that carry waits are kept.  Each deleted LDW saves ~100ns of PE
    sequencer dispatch."""
    n_del = 0
    for f in nc.m.functions:
        for bb in f.blocks:
            il = bb.instructions
            out = []
            pk = None
            changed = False
            for ins in il:
                tname = type(ins).__name__
                if tname == "InstLdweights":
                    si = ins.sync_info
                    has_sync = si is not None and (
                        len(si.on_wait) > 0 or len(si.on_update) > 0
                    )
                    key = (
                        str(ins.ins[0]),
                        str(ins.perf_mode),
                        str(ins.tile_position),
                        str(ins.is_transpose),
                    )
                    if key == pk and not has_sync:
                        n_del += 1
                        changed = True
                        continue
                    pk = key
                elif tname == "InstMatmult" and getattr(ins, "is_transpose", None):
                    pk = None  # transpose clobbers the loaded weights
                out.append(ins)
            if changed:
                il[:] = out
    return n_del


def _split_excess_waits(nc):
    """Workaround for this walrus build: engine (TPB) instructions accept at
    most one sync-wait command (EventSemaphore: two), but Tile attaches one
    wait per dependency.  Move excess waits onto same-engine NOPs inserted
    immediately before the over-limit instruction."""
    n_new = 0
    for f in nc.m.functions:
        for bb in f.blocks:
            il = bb.instructions
            out = []
            changed = False
            for ins in il:
                si = ins.sync_info
                tname = type(ins).__name__
                if si is not None:
                    cap = 2 if tname == "InstEventSemaphore" else 1
                    waits = list(si.on_wait)
                    if len(waits) > cap:
                        for w in waits[cap:]:
                            nop = mybir.InstNoOp(
                                name=f"I-wsplit-{n_new}",
                                sync_info=mybir.SyncInfo(
                                    on_wait=[w], on_update=[]
                                ),
                                bass_nofuse=True,
                                engine=ins.engine,
                            )
                            n_new += 1
                            out.append(nop)
                        si.on_wait = waits[:cap]
                        changed = True
                out.append(ins)
            if changed:
                il[:] = out
    return n_new


def _build_attention():
    """Per-core attention program: 4 heads of one batch, fp8 DoubleRow."""
    nc = bass.Bass()

    # activations pre-chunked on host: [sc, kp, 128, 2, 512] fp8 so each
    # (sc, kp) DMA piece is fully contiguous (fast descriptor generation)
    xq_t = nc.dram_tensor("xq_t", [NS, KP, 128, 2, 512], FP8, kind="ExternalInput")
    xk_t = nc.dram_tensor("xk_t", [NS, KP, 128, 2, 512], FP8, kind="ExternalInput")
    xv_t = nc.dram_tensor("xv_t", [NS, KP, 128, 2, 512], FP8, kind="ExternalInput")
    # weights pre-permuted+scaled on host: [128, kp, 2, 512] fp8
    wq = nc.dram_tensor("wq", [128, KP, 2, HW], FP8, kind="ExternalInput")
    wk = nc.dram_tensor("wk", [128, KP, 2, HW], FP8, kind="ExternalInput")
    wv = nc.dram_tensor("wv", [128, KP, 2, HW], FP8, kind="ExternalInput")
    # biases packed [bq32 | bk32 | bv_bcast]: [128, HG+HG+HW] f32
    bqkv = nc.dram_tensor("bqkv", [128, 2 * HG + HW], F32, kind="ExternalInput")
    # per-head NORMALIZED attention output O^T (softmax applied in-kernel)
    o_t = nc.dram_tensor("o_t", [HG, DK, S], BF16, kind="ExternalOutput")
    # DRAM scratch for the 1/rowsum partition-broadcast round trip (SBUF
    # sources cannot have stride-0 partition dims in DMA APs)
    rsd = nc.dram_tensor("rsd", [HG, S], F32, kind="Internal")

    with TileContext(nc) as tc:
        from contextlib import ExitStack

        with ExitStack() as ctx:
            consts = ctx.enter_context(tc.tile_pool(name="consts", bufs=1))
            xpool = ctx.enter_context(tc.tile_pool(name="x", bufs=1))
            wpool = ctx.enter_context(tc.tile_pool(name="w", bufs=1))
            proj_out = ctx.enter_context(tc.tile_pool(name="proj_out", bufs=1))
            ptpool = ctx.enter_context(tc.tile_pool(name="pt", bufs=2))
            osbpool = ctx.enter_context(tc.tile_pool(name="osb", bufs=4))
            ripool = ctx.enter_context(tc.tile_pool(name="ri", bufs=4))
            rbpool = ctx.enter_context(tc.tile_pool(name="rb", bufs=2))
            stpool = ctx.enter_context(
                tc.tile_pool(name="st", bufs=2, space="PSUM")
            )
            rspool = ctx.enter_context(
                tc.tile_pool(name="rsp", bufs=2, space="PSUM")
            )
            otpool = ctx.enter_context(
                tc.tile_pool(name="ot", bufs=1, space="PSUM")
            )

            # --- constants ---
            tril = consts.tile([128, 128], F32)  # additive: -1e9 where k > q
            make_lower_triangular(nc, tril, val=NEG_INF, diag=False)
            # pair-dim stride must be 16B-aligned for dual-fp8 LDWEIGHTS
            ones8 = consts.tile([128, 2, 16], FP8)
            nc.vector.memset(ones8, 1.0)
            negc_sb = consts.tile([128, 1], F32)
            nc.vector.memset(negc_sb, -float(C_SHIFT))
            zero_sb = consts.tile([128, 1], F32)
            nc.vector.memset(zero_sb, 0.0)

            b_sb = consts.tile([128, 2 * HG + HW], F32)
            # bias DMA split so the first K drain doesn't wait on one big DMA
            nc.scalar.dma_start(out=b_sb[:, 0:2 * HG], in_=bqkv[:, 0:2 * HG])
            nc.scalar.dma_start(
                out=b_sb[:, 2 * HG:2 * HG + 256], in_=bqkv[:, 2 * HG:2 * HG + 256]
            )
            nc.scalar.dma_start(
                out=b_sb[:, 2 * HG + 256:], in_=bqkv[:, 2 * HG + 256:]
            )
            bq_sb = b_sb[:, 0:HG]
            bk_sb = b_sb[:, HG:2 * HG]
            bv_sb = b_sb[:, 2 * HG:]

            # --- SBUF tiles for activations / projections ---
            xq8 = xpool.tile([128, NS, KP, 2, 512], FP8, tag="xq", name="xq")
            xk8 = xpool.tile([128, NS, KP, 2, 512], FP8, tag="xk", name="xk")
            xv8 = xpool.tile([128, NS, KP, 2, 512], FP8, tag="xv", name="xv")
            wq_t = wpool.tile([128, KP, 2, HW], FP8, tag="wq_t", name="wq_t")
            wk_t = wpool.tile([128, KP, 2, HW], FP8, tag="wk_t", name="wk_t")
            wv_t = wpool.tile([128, KP, 2, HW], FP8, tag="wv_t", name="wv_t")
            qt_sb = [proj_out.tile([128, S], BF16, tag=f"qt{h}", name=f"qt{h}")
                     for h in range(HG)]
            kt_sb = [proj_out.tile([128, S], BF16, tag=f"kt{h}", name=f"kt{h}")
                     for h in range(HG)]
            v8_sb = proj_out.tile([128, NJ, HW], FP8, tag="v8", name="v8")

            # --- input DMAs, kp-split for queue parallelism, spread across
            # issue engines so descriptor generation doesn't serialize ---
            # issue-engine split tuned so each engine's first compute op isn't
            # delayed by its DMA descriptor generation backlog (~0.6us per
            # dma_start).  Only SP/Activation/GpSimd can initiate DMAs.
            for kp in range(KP):
                nc.sync.dma_start(out=wk_t[:, kp], in_=wk[:, kp])
            for sc in range(NS):
                for kp in range(KP):
                    nc.sync.dma_start(out=xk8[:, sc, kp], in_=xk_t[sc, kp])
            for kp in range(KP):
                nc.scalar.dma_start(out=wq_t[:, kp], in_=wq[:, kp])
            for kp in range(KP):
                nc.scalar.dma_start(out=xq8[:, 0, kp], in_=xq_t[0, kp])
            for sc in range(1, NS):
                for kp in range(KP):
                    nc.gpsimd.dma_start(out=xq8[:, sc, kp], in_=xq_t[sc, kp])
            for kp in range(KP):
                nc.sync.dma_start(out=wv_t[:, kp], in_=wv[:, kp])
            for sc in range(NS):
                for kp in range(KP):
                    nc.sync.dma_start(out=xv8[:, sc, kp], in_=xv_t[sc, kp])

            # ---------- emission helpers ----------
            bases = [512 * (m // 2) for m in range(NM)]
            pts_h = {}
            rb_h = {}   # (h, r) -> broadcast 1/rowsum tile [128, 512] bf16

            def emit_proj_sc(h, sc, w_t, b_col, out_sb):
                """One 512-query chunk of a K^T/Q^T projection (kp inner, so
                it can start as soon as that sc's activations land)."""
                ps = rspool.tile([128, 512], F32, tag="rsp", name=f"pj{h}{sc}")
                for kp in range(KP):
                    nc.tensor.matmul(
                        ps,
                        lhsT=w_t[:, kp, :, h * DK:(h + 1) * DK],
                        rhs=xk8[:, sc, kp] if w_t is wk_t else xq8[:, sc, kp],
                        start=(kp == 0),
                        stop=(kp == KP - 1),
                        perf_mode=DR,
                    )
                nc.vector.tensor_scalar_add(
                    out=out_sb[:, sc * 512:(sc + 1) * 512],
                    in0=ps,
                    scalar1=b_col[:, h:h + 1],
                )

            def emit_proj_block(h, w_t, x_t8, b_col, out_sb):
                """K^T/Q^T projection for one head, kp outer (4 LDWs)."""
                pss = [stpool.tile([128, 1024], F32, tag="st", name=f"ps{h}")
                       for _ in range(2)]
                for kp in range(KP):
                    for sc in range(4):
                        nc.tensor.matmul(
                            pss[sc // 2][:, (sc % 2) * 512:(sc % 2 + 1) * 512],
                            lhsT=w_t[:, kp, :, h * DK:(h + 1) * DK],
                            rhs=x_t8[:, sc, kp],
                            start=(kp == 0),
                            stop=(kp == KP - 1),
                            perf_mode=DR,
                        )
                for st2 in range(2):
                    nc.vector.tensor_scalar_add(
                        out=out_sb[:, st2 * 1024:(st2 + 1) * 1024],
                        in0=pss[st2],
                        scalar1=b_col[:, h:h + 1],
                    )

            def v_unit(sc, tt):
                """V projection chunk: V[sc queries, cols] for one 256-row
                slab pair; fp8 output at true scale."""
                ps = stpool.tile([128, 1024], F32, tag="st", name="psv")
                for sbl in range(2):
                    sbl2 = 2 * tt + sbl
                    for kp in range(KP):
                        nc.tensor.matmul(
                            ps[:, sbl * 512:(sbl + 1) * 512],
                            lhsT=xv8[:, sc, kp, :,
                                     sbl2 * 128:(sbl2 + 1) * 128],
                            rhs=wv_t[:, kp],
                            start=(kp == 0),
                            stop=(kp == KP - 1),
                            perf_mode=DR,
                        )
                for sbl in range(2):
                    sb = 4 * sc + 2 * tt + sbl
                    nc.vector.scalar_tensor_tensor(
                        out=v8_sb[:, sb, :],
                        in0=ps[:, sbl * 512:(sbl + 1) * 512],
                        scalar=1.0 / WS,
                        in1=bv_sb,
                        op0=mybir.AluOpType.mult,
                        op1=mybir.AluOpType.add,
                    )

            def emit_pts_alloc(h):
                pts = []
                for m in range(NM):
                    pt = ptpool.tile([128, 2, S - bases[m]], FP8,
                                     tag=f"ptp{m}", name=f"pt{h}_{m}")
                    pts.append(pt)
                pts_h[h] = pts
                # zero the causally-invalid diagonal blocks so DoubleRow
                # P@V / rowsum matmuls can run unmasked over full pairs
                # (on GpSimd to keep the DVE queue clear)
                for r in range(NS):
                    nc.gpsimd.memset(pts[2 * r][:, 1, 0:128], 0.0)
                    nc.gpsimd.memset(pts[2 * r + 1][:, :, 0:384], 0.0)

            def a_unit(h, j, hl):
                """Scores for key chunk j, query cols [hl*1024,(hl+1)*1024),
                plus the exp into fp8 P^T."""
                pts = pts_h[h]
                m = j // 2
                jq = j * 128
                base = bases[m]
                qlo = max(hl * 1024, jq)
                a = qlo - hl * 1024
                r0 = j // 4
                st = stpool.tile([128, 1024], F32, tag="st", name="st")
                for r in range(max(2 * hl, r0), 2 * hl + 2):
                    rqlo = max(r * 512, jq)
                    ra = rqlo - hl * 1024
                    nc.tensor.matmul(
                        st[:, ra:(r + 1) * 512 - hl * 1024],
                        lhsT=kt_sb[h][:, jq:jq + 128],
                        rhs=qt_sb[h][:, rqlo:(r + 1) * 512],
                        start=True,
                        stop=True,
                    )
                if qlo == jq:
                    nc.vector.tensor_add(
                        out=st[:, a:a + 128],
                        in0=st[:, a:a + 128],
                        in1=tril,
                    )
                nc.scalar.activation(
                    out=pts[m][:, j % 2, qlo - base:(hl + 1) * 1024 - base],
                    in_=st[:, a:1024],
                    func=mybir.ActivationFunctionType.Exp,
                    scale=float(SCALE / (WS * WS)),
                    bias=negc_sb,
                )

            def rs_unit(h, r):
                """Rowsums for query range r (fp8 DoubleRow ones-matmul over
                all P chunks), then 1/rowsum broadcast across partitions so
                the O^T drain can normalize in place."""
                pts = pts_h[h]
                rsp_pk = rspool.tile([128, 512], F32, tag="rsp",
                                     name=f"rsp{h}_{r}")
                for m in range(2 * r + 2):
                    nc.tensor.matmul(
                        rsp_pk[0:1, :],
                        lhsT=ones8[:, :, 0:1],
                        rhs=pts[m][:, :, r * 512 - bases[m]:
                                   (r + 1) * 512 - bases[m]],
                        start=(m == 0),
                        stop=(m == 2 * r + 1),
                        perf_mode=DR,
                    )
                # 1/rowsum = Exp(-Ln(rowsum)) on ScalarE (DVE's reciprocal is
                # a ~3.3us multipass op and clogs the vector queue; the DVE
                # custom-op fast reciprocal doesn't compile on this walrus
                # build).  Then partition-broadcast via a DRAM round trip
                # (SBUF DMA sources cannot have stride-0 partition dims).
                lntmp = ripool.tile([1, 512], F32, tag="ln", name=f"ln{h}_{r}")
                nc.scalar.activation(
                    out=lntmp,
                    in_=rsp_pk[0:1, :],
                    func=mybir.ActivationFunctionType.Ln,
                    bias=zero_sb[0:1, :],
                )
                ri = ripool.tile([1, 512], F32, tag="ri", name=f"ri{h}_{r}")
                nc.scalar.activation(
                    out=ri,
                    in_=lntmp,
                    func=mybir.ActivationFunctionType.Exp,
                    scale=-1.0,
                    bias=zero_sb[0:1, :],
                )
                rs_slice = rsd[h:h + 1, r * 512:(r + 1) * 512]
                d_wr = nc.sync.dma_start(out=rs_slice, in_=ri)
                rb = rbpool.tile([128, 512], F32, tag=f"rb{r}",
                                 name=f"rb{h}_{r}")
                for cs in range(2):
                    d_rd = nc.sync.dma_start(
                        out=rb[:, cs * 256:(cs + 1) * 256],
                        in_=_bcast_rows(rsd[h:h + 1, r * 512 + cs * 256:
                                            r * 512 + (cs + 1) * 256]),
                    )
                    _adh(d_rd.ins, d_wr.ins, reason="rowsum bcast RAW via DRAM")
                rb_h[(h, r)] = rb

            def b_units(h, halves=(0, 1)):
                """P@V for head h as a list of (cost_ns, closure) filler
                units: per-m matmul groups + normalized drains."""
                pts = pts_h[h]
                units = []
                state = {}

                def mk_group(half, m):
                    def fn():
                        if m == 0:
                            state[half] = otpool.tile(
                                [128, 1024], F32, tag="ot", name=f"ot{h}_{half}"
                            )
                        ot_ps = state[half]
                        for k in range(2):
                            r = 2 * half + k
                            if m >= 2 * r + 2:
                                continue
                            nc.tensor.matmul(
                                ot_ps[:, k * 512:(k + 1) * 512],
                                lhsT=v8_sb[:, 2 * m:2 * m + 2,
                                           h * DK:(h + 1) * DK],
                                rhs=pts[m][:, :, r * 512 - bases[m]:
                                           (r + 1) * 512 - bases[m]],
                                start=(m == 0),
                                stop=(m == 2 * r + 1),
                                perf_mode=DR,
                            )
                    return fn

                def mk_drain(half):
                    def fn():
                        ot_ps = state[half]
                        for k in range(2):
                            r = 2 * half + k
                            o_sb = osbpool.tile([128, 512], BF16, tag="osb",
                                                name=f"o_sb{h}_{r}")
                            with nc.allow_low_precision(
                                reason="bf16 attn out: ~0.4% vs 2e-2 gate"
                            ):
                                nc.vector.tensor_mul(
                                    out=o_sb, in0=ot_ps[:, k * 512:(k + 1) * 512],
                                    in1=rb_h.pop((h, r)),
                                )
                            for cs in range(2):
                                nc.gpsimd.dma_start(
                                    out=o_t[h, :, r * 512 + cs * 256:
                                            r * 512 + (cs + 1) * 256],
                                    in_=o_sb[:, cs * 256:(cs + 1) * 256],
                                )
                    return fn

                for half in halves:
                    nm_half = 2 * (2 * half + 1) + 2
                    for m in range(nm_half):
                        n_mm = sum(1 for k in range(2)
                                   if m < 2 * (2 * half + k) + 2)
                        units.append((150 + 213 * n_mm + 135, mk_group(half, m)))
                    units.append((100, mk_drain(half)))
                return units

            # ---------- emission schedule ----------
            # K0/Q0 projections in sc-chunks so the PE starts as soon as the
            # first kp-split DMAs land, and A(0) can begin before xv arrives.
            for sc in range(NS):
                emit_proj_sc(0, sc, wk_t, bk_sb, kt_sb[0])
                emit_proj_sc(0, sc, wq_t, bq_sb, qt_sb[0])

            for h in range(HG - 1):
                emit_pts_alloc(h)
                # filler units woven between A chunks: V projection during
                # head 0, P@V of head h-1 afterwards
                if h == 0:
                    fillers = [(2800, (lambda sc=sc, tt=tt:
                                       v_unit(sc, tt)))
                               for sc in range(NS) for tt in range(2)]
                else:
                    fillers = b_units(h - 1)
                total_fill = sum(c for c, _ in fillers) or 1
                fillers = list(fillers)
                balance = 0.0
                emitted = 0.0
                # ScalarE cost of each A unit, to pace fillers linearly
                a_units = []
                for j in range(NJ):
                    for hl in range(j // 8, 2):
                        qlo = max(hl * 1024, j * 128)
                        a_units.append((j, hl, (1024 * (hl + 1) - qlo) * 0.72
                                        + 260))
                total_a = sum(c for _, _, c in a_units)
                # head 0's fillers (V projection) wait on the xv DMAs, which
                # are last in the input queue — delay them into the window
                ramp = 0.25 if h == 0 else 0.0
                for j, hl, cost in a_units:
                    a_unit(h, j, hl)
                    emitted += cost
                    # rowsums as soon as all P chunks for range r are exp'd
                    if hl == 1 and (j + 1) % 4 == 0:
                        rs_unit(h, (j + 1) // 4 - 1)
                    want = total_fill * max(0.0, emitted / total_a - ramp) \
                        / (1.0 - ramp)
                    while fillers and balance < want:
                        c, fn = fillers.pop(0)
                        fn()
                        balance += c
                for c, fn in fillers:
                    fn()
                emit_proj_block(h + 1, wk_t, xk8, bk_sb, kt_sb[h + 1])
                emit_proj_block(h + 1, wq_t, xq8, bq_sb, qt_sb[h + 1])

            # --- head 3: hl-major order so the upper query halves (and their
            # rowsums) finish early, letting B(3) half1 start before the
            # window ends; the final rowsum-broadcast chains overlap leftover
            # B(2)/B(3) matmul groups instead of draining after everything ---
            h = HG - 1
            emit_pts_alloc(h)
            fillers = b_units(h - 1)
            total_a1 = 16.0
            balance = 0.0
            emitted = 0.0
            total_fill = sum(c for c, _ in fillers) or 1
            for j in range(NJ):
                a_unit(h, j, 1)
                emitted += 1.0
                if j == 11:
                    rs_unit(h, 2)
                if j == 15:
                    rs_unit(h, 3)
                want = total_fill * emitted / total_a1
                while fillers and balance < want:
                    c, fn = fillers.pop(0)
                    fn()
                    balance += c
            for c, fn in fillers:
                fn()
            # B(3)'s upper half interleaves with the remaining lower-half
            # score chunks; its lower half (and the rowsum chains for query
            # ranges 0-1) drain at the very end
            b3h1 = b_units(h, halves=(1,))
            b3h0 = b_units(h, halves=(0,))
            for j in range(8):
                a_unit(h, j, 0)
                if j == 3:
                    rs_unit(h, 0)
                if j == 7:
                    rs_unit(h, 1)
                if b3h1:
                    c, fn = b3h1.pop(0)
                    fn()
            for c, fn in b3h1:
                fn()
            for c, fn in b3h0:
                fn()
    _dedupe_ldweights(nc)
    _split_excess_waits(nc)
    return nc


def _build_layernorm(affine=True):
    """Per-core: residual add + LayerNorm over 1024 rows of [8192, 1024].

    Inputs arrive bf16 and already normalized (softmax applied in the
    attention kernel).  Everything stays bf16 so DVE runs in 2x mode.
    affine=False omits gamma/beta (valid when gamma==1, beta==0)."""
    nc = bass.Bass()
    RPC = (B * S) // NCORES  # 1024 rows per core

    attn = nc.dram_tensor("attn", [RPC, D], BF16, kind="ExternalInput")
    resid = nc.dram_tensor("resid", [RPC, D], BF16, kind="ExternalInput")
    gamma = nc.dram_tensor("gamma", [D], F32, kind="ExternalInput")
    beta = nc.dram_tensor("beta", [D], F32, kind="ExternalInput")
    out = nc.dram_tensor("out", [RPC, D], BF16, kind="ExternalOutput")

    with TileContext(nc) as tc:
        with (
            tc.tile_pool(name="consts", bufs=1) as consts,
            tc.tile_pool(name="work", bufs=3) as work,
            tc.tile_pool(name="stat", bufs=4) as statp,
        ):
            if affine:
                gamma_sb = consts.tile([128, D], BF16)
                beta_sb = consts.tile([128, D], BF16)
                nc.gpsimd.dma_start(
                    out=gamma_sb,
                    in_=bass.AP(tensor=gamma[:].tensor, offset=gamma[:].offset,
                                ap=[[0, 128]] + list(gamma[:].ap)),
                )
                nc.gpsimd.dma_start(
                    out=beta_sb,
                    in_=bass.AP(tensor=beta[:].tensor, offset=beta[:].offset,
                                ap=[[0, 128]] + list(beta[:].ap)),
                )
            eps_sb = consts.tile([128, 1], F32)
            nc.vector.memset(eps_sb, EPS)

            nsub = D // 512  # bn_stats free-dim limit
            NT = RPC // 128
            for t in range(NT):
                ab = work.tile([128, D], BF16, tag="ab", name="ab")
                rb = work.tile([128, D], BF16, tag="rb", name="rb")
                x = work.tile([128, D], BF16, tag="x", name="x")
                nc.sync.dma_start(out=ab, in_=attn[t * 128:(t + 1) * 128, :])
                nc.sync.dma_start(out=rb, in_=resid[t * 128:(t + 1) * 128, :])
                with nc.allow_low_precision(
                    reason="bf16 residual add: ~0.4% vs 2e-2 gate"
                ):
                    nc.vector.tensor_add(out=x, in0=ab, in1=rb)

                stats = statp.tile([128, nsub, 6], F32, tag="stats",
                                   name="stats")
                for sgi in range(nsub):
                    nc.vector.bn_stats(
                        out=stats[:, sgi, :],
                        in_=x[:, sgi * 512:(sgi + 1) * 512],
                    )
                mv = statp.tile([128, 2], F32, tag="mv", name="mv")
                nc.vector.bn_aggr(out=mv, in_=stats)
                rstd = statp.tile([128, 1], F32, tag="rstd", name="rstd")
                nc.scalar.activation(
                    out=rstd,
                    in_=mv[:, 1:2],
                    func=mybir.ActivationFunctionType.Sqrt,
                    bias=eps_sb,
                    scale=1.0,
                )
                nc.vector.reciprocal(out=rstd, in_=rstd)
                xo = work.tile([128, D], BF16, tag="xo", name="xo")
                with nc.allow_low_precision(
                    reason="bf16 LN output: ~0.2% rounding vs 2e-2 gate"
                ):
                    nc.vector.tensor_scalar(
                        out=xo if not affine else x,
                        in0=x,
                        scalar1=mv[:, 0:1],
                        scalar2=rstd,
                        op0=mybir.AluOpType.subtract,
                        op1=mybir.AluOpType.mult,
                    )
                    if affine:
                        nc.vector.tensor_mul(out=x, in0=x, in1=gamma_sb)
                        nc.vector.tensor_add(out=xo, in0=x, in1=beta_sb)
                nc.gpsimd.dma_start(
                    out=out[t * 128:(t + 1) * 128, :], in_=xo
                )
    _split_excess_waits(nc)
    return nc


_CACHE = {}


def _get_programs(affine=True):
    # note: walrus's --enable-ldw-opt=true rejects DoubleRow LDWEIGHTS
    # ("InstLdweights is not compatible with LDW optimization"), so redundant
    # weight loads are removed by _dedupe_ldweights instead.
    if "attn" not in _CACHE:
        _CACHE["attn"] = _build_attention()
    key = ("ln", affine)
    if key not in _CACHE:
        _CACHE[key] = _build_layernorm(affine=affine)
    return _CACHE["attn"], _CACHE[key]


def _prep_x(xb):
    """[S, D] f32 -> [sc, kp, 128, 2, 512] fp8 with
    x8[sc, kp, p, i, s'] = X[sc*512+s', 256*kp+128*i+p]."""
    xT = np.asarray(xb, dtype=np.float32).T  # [D, S]
    arr = xT.reshape(KP, 2, 128, NS, 512).transpose(3, 0, 2, 1, 4)
    return np.ascontiguousarray(arr.astype(NPFP8))


def _prep_w(Wm, g):
    """W[:, g*512:(g+1)*512]*WS -> [128, kp, 2, 512] fp8."""
    ws = np.asarray(Wm, dtype=np.float32)[:, g * HW:(g + 1) * HW] * WS
    arr = ws.reshape(KP, 2, 128, HW).transpose(2, 0, 1, 3)
    return np.ascontiguousarray(arr.astype(NPFP8))


def _run(inputs, trace=False):
    """Returns (output, attn_results, ln_results)."""
    gamma_np = np.asarray(inputs["gamma"], dtype=np.float32)
    beta_np = np.asarray(inputs["beta"], dtype=np.float32)
    affine = not (np.all(gamma_np == 1.0) and np.all(beta_np == 0.0))
    nc_attn, nc_ln = _get_programs(affine=affine)

    q = np.ascontiguousarray(np.asarray(inputs["queries"], dtype=np.float32))
    k = np.ascontiguousarray(np.asarray(inputs["keys"], dtype=np.float32))
    v = np.ascontiguousarray(np.asarray(inputs["values"], dtype=np.float32))
    Wq = np.asarray(inputs["Wq"], dtype=np.float32)
    Wk = np.asarray(inputs["Wk"], dtype=np.float32)
    Wv = np.asarray(inputs["Wv"], dtype=np.float32)
    bq = np.asarray(inputs["bq"], dtype=np.float32)
    bk = np.asarray(inputs["bk"], dtype=np.float32)
    bv = np.asarray(inputs["bv"], dtype=np.float32)

    xt = {}
    for b in range(B):
        xt[("q", b)] = _prep_x(q[b])
        xt[("k", b)] = _prep_x(k[b])
        xt[("v", b)] = _prep_x(v[b])
    wslices = {}
    bslices = {}
    for g in range(2):
        cols = slice(g * HW, (g + 1) * HW)
        wslices[("q", g)] = _prep_w(Wq, g)
        wslices[("k", g)] = _prep_w(Wk, g)
        wslices[("v", g)] = _prep_w(Wv, g)
        bslices[g] = np.ascontiguousarray(np.concatenate([
            (WS * bq[cols]).reshape(HG, 128).T,
            (WS * bk[cols]).reshape(HG, 128).T,
            np.broadcast_to(bv[cols], (128, HW)),
        ], axis=1, dtype=np.float32))

    in_maps = []
    for c in range(NCORES):
        b, g = c // 2, c % 2
        in_maps.append({
            "xq_t": xt[("q", b)],
            "xk_t": xt[("k", b)],
            "xv_t": xt[("v", b)],
            "wq": wslices[("q", g)],
            "wk": wslices[("k", g)],
            "wv": wslices[("v", g)],
            "bqkv": bslices[g],
        })

    res1 = run_bass_kernel_spmd(
        nc_attn, in_maps, core_ids=list(range(NCORES)), trace=trace
    )

    # assemble attention output [B, S, D] bf16 (already normalized in-kernel)
    attn_full = np.empty((B, S, D), dtype=NPBF16)
    for c in range(NCORES):
        b, g = c // 2, c % 2
        ot = res1.results[c]["o_t"]  # [HG, DK, S]
        for i in range(HG):
            attn_full[b, :, (g * HG + i) * DK:(g * HG + i + 1) * DK] = ot[i].T

    attn_flat = attn_full.reshape(B * S, D)
    q_flat = q.reshape(B * S, D).astype(NPBF16)
    RPC = (B * S) // NCORES
    in_maps2 = []
    for c in range(NCORES):
        rows = slice(c * RPC, (c + 1) * RPC)
        in_maps2.append({
            "attn": np.ascontiguousarray(attn_flat[rows]),
            "resid": np.ascontiguousarray(q_flat[rows]),
            "gamma": gamma_np,
            "beta": beta_np,
        })
    res2 = run_bass_kernel_spmd(
        nc_ln, in_maps2, core_ids=list(range(NCORES)), trace=trace
    )
    out = np.concatenate(
        [res2.results[c]["out"].astype(np.float32) for c in range(NCORES)],
        axis=0,
    ).reshape(B, S, D)
    return out, res1, res2


def kernel(**inputs):
    out, _, _ = _run(inputs, trace=False)
    return out
